# revision 14
# baseline (speedup 1.0000x reference)
"""Trainium2 Bass kernel for a multi-head ReLU-attention transformer layer.

Shapes (hardcoded): B=32, F=1024, DIN=64, DOUT=64, H=4.
  qkv   = einsum("bfi,hkio->bhkfo", x, Wqkv)
  scores= relu(q @ k^T / sqrt(DOUT))
  head  = scores @ v
  out   = LN(concat(head) @ Wo + bo + x) * gamma + beta

Sharding: pure data-parallel over batch B across 8 NeuronCores (4 b/core).

Host-side algebraic folds (exact or fp32-precise):
  - 1/sqrt(DOUT)=0.125 folded into Wq (exact, power of two).
  - Wo folded into Wv:  proj = sum_h scores_h @ (Wv_h @ Wo_h).

Per-batch device pipeline (all matmuls bf16 with fp32 PSUM accumulation —
fp32/fp32r matmuls silently return zeros on this toolchain):
  x -> (bf16 cast, DMA-xbar transpose) xT, duplicated onto both partition
  halves so 64-deep contractions pack two-per-MM via PE row groups.
  Q^T/K^T per head-pair land stacked on partition halves; scoresT =
  relu(K^T_tile^T @ Q^T) drains PSUM->SBUF via ScalarE/VectorE (the
  bandwidth-critical path: PSUM fp32 reads are capped at 1 elem/lane/cycle);
  projT accumulates over heads and g-tiles into two [64,512] PSUM banks
  (matmul PSUM outputs must be bank-aligned on this hardware); DMA-xbar
  transposes back to natural layout; residual + LayerNorm in fp32; DMA out.

This walrus build accepts only ONE sync wait per instruction; Tile emits
multi-waits, so split_multiwaits() hoists extras onto NoOps post-schedule.
"""

import numpy as np

import concourse.bass as bass
import concourse.mybir as mybir
import concourse.tile as tile
from concourse.bass_utils import run_bass_kernel_spmd


def split_multiwaits(nc):
    """Hoist all but the last sync wait of any instruction onto standalone
    NoOps inserted just before it on the same engine — semantically identical
    (same-engine program order runs the waits first), but keeps every
    instruction within this walrus build's one-wait limit."""
    n_split = 0
    max_upd = 0

    def fix_block(bl):
        nonlocal n_split, max_upd
        insts = list(bl.instructions)
        out = []
        changed = False
        for inst in insts:
            si = inst.sync_info
            if si is not None:
                max_upd = max(max_upd, len(si.on_update))
                waits = list(si.on_wait)
                if len(waits) > 1:
                    for k, w in enumerate(waits[:-1]):
                        nop = mybir.InstNoOp(
                            name=f"{inst.name}-wsplit{k}", ins=[], outs=[])
                        nop.engine = inst.engine
                        nop.sync_info = mybir.SyncInfo(
                            on_wait=[w], on_update=[])
                        out.append(nop)
                    inst.sync_info = mybir.SyncInfo(
                        on_wait=[waits[-1]], on_update=list(si.on_update))
                    n_split += 1
                    changed = True
            out.append(inst)
        if changed:
            bl.instructions = out
        for sub in getattr(bl, "blocks", None) or []:
            fix_block(sub)

    for f in nc.m.functions:
        for bl in f.blocks:
            fix_block(bl)
    assert max_upd <= 1, f"need update-splitting too: {max_upd}"
    return n_split


B, F, DIN, DOUT, H = 32, 1024, 64, 64, 4
NCORES = 8
BPC = B // NCORES  # batches per core
NT = F // 128  # 8 f-tiles per batch
FP32 = mybir.dt.float32
BF16 = mybir.dt.bfloat16
EPS = 1e-5

_cache = {}


def _build(use_gb: bool, use_bo: bool, stage: int = 99):
    nc = bass.Bass("TRN2", target_bir_lowering=False, debug=False,
                   num_devices=NCORES)
    x_d = nc.dram_tensor("x", [BPC, F, DIN], FP32, kind="ExternalInput").ap()
    wq_d = nc.dram_tensor("wq", [128, 128], BF16, kind="ExternalInput").ap()
    wk_d = nc.dram_tensor("wk", [128, 128], BF16, kind="ExternalInput").ap()
    wv_d = nc.dram_tensor("wv", [128, 256], BF16, kind="ExternalInput").ap()
    if use_gb:
        gb_d = nc.dram_tensor("gb", [2, DIN], FP32, kind="ExternalInput").ap()
    if use_bo:
        bo_d = nc.dram_tensor("bo", [DIN], FP32, kind="ExternalInput").ap()
    y_d = nc.dram_tensor("y", [BPC, F, DIN], FP32, kind="ExternalOutput").ap()

    # alternate score drains between ScalarE and VectorE, weighted toward
    # ScalarE (1.2 GHz vs 0.96): pattern gives ACT 3 of every 5
    drain_pat = [True, False, True, False, True]
    drain_i = [0]

    def drain_relu(out_ap, in_ap):
        use_act = drain_pat[drain_i[0] % len(drain_pat)]
        drain_i[0] += 1
        if use_act:
            nc.scalar.activation(out=out_ap, in_=in_ap,
                                 func=mybir.ActivationFunctionType.Relu)
        else:
            nc.vector.tensor_scalar_max(out=out_ap, in0=in_ap, scalar1=0.0)

    def drain_copy(out_ap, in_ap, act):
        if act:
            nc.scalar.activation(out=out_ap, in_=in_ap,
                                 func=mybir.ActivationFunctionType.Copy)
        else:
            nc.vector.tensor_copy(out=out_ap, in_=in_ap)

    with tile.TileContext(nc) as tc:
        with (
            tc.tile_pool(name="const", bufs=1) as constp,
            tc.tile_pool(name="xp", bufs=2) as xp,
            tc.tile_pool(name="xtp", bufs=2) as xtp,
            tc.tile_pool(name="qkp", bufs=2) as qkp,
            tc.tile_pool(name="vp", bufs=2) as vp,
            tc.tile_pool(name="scp", bufs=8) as scp,
            tc.tile_pool(name="pjp", bufs=2) as pjp,
            tc.tile_pool(name="resp", bufs=2) as resp,
            tc.tile_pool(name="statp", bufs=4) as statp,
            tc.tile_pool(name="mm", bufs=3, space="PSUM") as psmm,
            tc.tile_pool(name="acc", bufs=2, space="PSUM") as psacc,
        ):
            # ---- constants ----
            eps_sb = constp.tile([128, 1], FP32)
            nc.vector.memset(eps_sb, EPS)
            wq_sb = constp.tile([128, 128], BF16)
            nc.sync.dma_start(out=wq_sb, in_=wq_d)
            wk_sb = constp.tile([128, 128], BF16)
            nc.sync.dma_start(out=wk_sb, in_=wk_d)
            wv_sb = constp.tile([128, 256], BF16)
            nc.sync.dma_start(out=wv_sb, in_=wv_d)
            if use_gb:
                g_rep = constp.tile([128, NT, DIN], FP32)
                b_rep = constp.tile([128, NT, DIN], FP32)
                for t in range(NT):
                    nc.gpsimd.dma_start(
                        out=g_rep[:, t, :],
                        in_=bass.AP(gb_d.tensor, 0, [[0, 128], [1, DIN]]))
                    nc.gpsimd.dma_start(
                        out=b_rep[:, t, :],
                        in_=bass.AP(gb_d.tensor, DIN, [[0, 128], [1, DIN]]))
            if use_bo:
                bo_rep = constp.tile([128, DIN], FP32)
                nc.gpsimd.dma_start(
                    out=bo_rep,
                    in_=bass.AP(bo_d.tensor, 0, [[0, 128], [1, DIN]]))

            for b in range(BPC):
                # ---- load x (natural: partition = f within tile) ----
                x_sb = xp.tile([128, NT, DIN], FP32, tag="x")
                nc.sync.dma_start(
                    out=x_sb, in_=x_d[b].rearrange("(t p) j -> p t j", p=128))
                if use_bo:
                    x_res = xp.tile([128, NT, DIN], FP32, tag="xres")
                    for t in range(NT):
                        nc.vector.tensor_add(
                            out=x_res[:, t, :], in0=x_sb[:, t, :], in1=bo_rep)
                else:
                    x_res = x_sb
                x_bf = xp.tile([128, NT, DIN], BF16, tag="xbf")
                nc.vector.tensor_copy(out=x_bf, in_=x_sb)

                # ---- transpose x -> xT [64, 1024] via DMA xbar, dup ----
                # xbar tiles are 16x128, so transpose f-tile PAIRS as
                # [128,128] blocks: top half = xT of even tile, bottom = odd.
                xt = xtp.tile([128, F], BF16, tag="xt")
                for u in range(NT // 2):
                    tmp = xtp.tile([128, 128], BF16, tag="tmpt",
                                   name=f"tmp{u}_{b}")
                    nc.sync.dma_start_transpose(
                        out=tmp,
                        in_=x_bf[:, 2 * u:2 * u + 2, :].rearrange(
                            "p t j -> p (t j)"))
                    nc.sync.dma_start(
                        out=xt[0:64, bass.ts(2 * u, 128)], in_=tmp[0:64, :])
                    nc.sync.dma_start(
                        out=xt[0:64, bass.ts(2 * u + 1, 128)],
                        in_=tmp[64:128, :])
                nc.sync.dma_start(out=xt[64:128, :], in_=xt[0:64, :])

                if stage < 2:
                    nc.sync.dma_start(
                        out=y_d[b].rearrange("(t p) j -> p t j", p=128),
                        in_=x_sb)
                    continue
                # ---- QKV projections (row-packed pairs) ----
                qk_sb = []
                for w_sb, nm in ((wq_sb, "q"), (wk_sb, "k")):
                    ps_a = psmm.tile([128, 2, 512], FP32, tag="mm")
                    ps_b = psmm.tile([128, 2, 512], FP32, tag="mm")
                    for fc in range(2):
                        fsl = bass.ts(fc, 512)
                        nc.tensor.matmul(
                            ps_a[:, fc, :], w_sb[0:64, :],
                            xt[0:64, fsl], start=True, stop=True)
                        nc.tensor.matmul(
                            ps_b[:, fc, :], w_sb[64:128, :],
                            xt[64:128, fsl], start=True, stop=True)
                    sb_a = qkp.tile([128, F], BF16, tag=nm + "a")
                    sb_b = qkp.tile([128, F], BF16, tag=nm + "b")
                    drain_copy(sb_a.rearrange("p (c f) -> p c f", c=2), ps_a,
                               act=True)
                    drain_copy(sb_b.rearrange("p (c f) -> p c f", c=2), ps_b,
                               act=False)
                    qk_sb.append((sb_a, sb_b))
                (qt_a, qt_b), (kt_a, kt_b) = qk_sb

                if stage < 3:
                    nc.sync.dma_start(
                        out=y_d[b].rearrange("(t p) j -> p t j", p=128),
                        in_=x_sb)
                    continue
                # v' = x @ (Wv@Wo): natural [g, (h o)=256], g-tile pairs
                # packed via row groups; one MM per PSUM bank (bank-aligned)
                vt = vp.tile([128, NT, 256], BF16, tag="v")
                for pair in range(4):
                    v_ps = psmm.tile([128, 2, 512], FP32, tag="mm",
                                     name=f"v_ps{pair}_{b}")
                    gt0 = 2 * pair
                    nc.tensor.matmul(
                        v_ps[:, 0, 0:256], xt[0:64, bass.ts(gt0, 128)],
                        wv_sb[0:64, :], start=True, stop=True)
                    nc.tensor.matmul(
                        v_ps[:, 1, 0:256], xt[64:128, bass.ts(gt0 + 1, 128)],
                        wv_sb[64:128, :], start=True, stop=True)
                    drain_copy(vt[:, bass.ts(pair, 2), :], v_ps[:, :, 0:256],
                               act=(pair % 2 == 0))

                if stage < 4:
                    nc.sync.dma_start(
                        out=y_d[b].rearrange("(t p) j -> p t j", p=128),
                        in_=x_sb)
                    continue
                # ---- attention: scoresT then projT accumulation ----
                # projT f-chunk accumulators: [j=64, f 512] each, partition 0
                out_f = [psacc.tile([64, 512], FP32, tag="acc",
                                    name=f"out_f{fc}_{b}")
                         for fc in range(2)]
                for hp in range(2):
                    qt = qt_a if hp == 0 else qt_b
                    kt = kt_a if hp == 0 else kt_b
                    for gt in range(NT):
                        gsl = bass.ts(gt, 128)
                        s0_ps = psmm.tile([128, 2, 512], FP32, tag="mm")
                        s1_ps = psmm.tile([128, 2, 512], FP32, tag="mm")
                        for fc in range(2):
                            fsl = bass.ts(fc, 512)
                            nc.tensor.matmul(
                                s0_ps[:, fc, :], kt[0:64, gsl],
                                qt[0:64, fsl], start=True, stop=True)
                            nc.tensor.matmul(
                                s1_ps[:, fc, :], kt[64:128, gsl],
                                qt[64:128, fsl], start=True, stop=True)
                        sc0 = scp.tile([128, 2, 512], BF16, tag="sc")
                        sc1 = scp.tile([128, 2, 512], BF16, tag="sc")
                        drain_relu(sc0, s0_ps)
                        drain_relu(sc1, s1_ps)
                        first = hp == 0 and gt == 0
                        last = hp == 1 and gt == NT - 1
                        for fc in range(2):
                            nc.tensor.matmul(
                                out_f[fc][:, :],
                                vt[:, gt, bass.ts(2 * hp, 64)],
                                sc0[:, fc, :],
                                start=first, stop=False,
                                skip_group_check=True)
                            nc.tensor.matmul(
                                out_f[fc][:, :],
                                vt[:, gt, bass.ts(2 * hp + 1, 64)],
                                sc1[:, fc, :],
                                start=False, stop=last,
                                skip_group_check=True)

                if stage < 5:
                    nc.sync.dma_start(
                        out=y_d[b].rearrange("(t p) j -> p t j", p=128),
                        in_=x_sb)
                    continue
                # ---- projT -> natural + residual + LayerNorm ----
                pj = pjp.tile([64, 2, 512], BF16, tag="pj")
                drain_copy(pj[:, 0, :], out_f[0], act=False)
                drain_copy(pj[:, 1, :], out_f[1], act=True)
                nat_sb = resp.tile([128, NT, DIN], BF16, tag="natsb")
                for t in range(NT):
                    fc, tw = divmod(t, 4)
                    nc.sync.dma_start_transpose(
                        out=nat_sb[:, t, :], in_=pj[:, fc, bass.ts(tw, 128)])
                res = resp.tile([128, NT, DIN], FP32, tag="res")
                nc.vector.tensor_add(out=res, in0=nat_sb, in1=x_res)

                sq = resp.tile([128, NT, DIN], FP32, tag="sq")
                nc.gpsimd.tensor_mul(out=sq, in0=res, in1=res)
                stat = statp.tile([128, NT, 2], FP32, tag="stat")
                nc.vector.tensor_reduce(
                    out=stat[:, :, 0], in_=res,
                    axis=mybir.AxisListType.X, op=mybir.AluOpType.add)
                nc.vector.tensor_reduce(
                    out=stat[:, :, 1], in_=sq,
                    axis=mybir.AxisListType.X, op=mybir.AluOpType.add)
                mv = statp.tile([128, NT, 4], FP32, tag="mv")
                # mean, E[x^2]
                nc.vector.tensor_scalar_mul(
                    out=mv[:, :, 0], in0=stat[:, :, 0], scalar1=1.0 / DIN)
                nc.vector.tensor_scalar_mul(
                    out=mv[:, :, 1], in0=stat[:, :, 1], scalar1=1.0 / DIN)
                # var = E[x^2] - mean^2
                nc.vector.tensor_mul(
                    out=mv[:, :, 2], in0=mv[:, :, 0], in1=mv[:, :, 0])
                nc.vector.tensor_sub(
                    out=mv[:, :, 2], in0=mv[:, :, 1], in1=mv[:, :, 2])
                # rstd = 1/sqrt(var + eps)
                nc.scalar.activation(
                    out=mv[:, :, 3], in_=mv[:, :, 2],
                    func=mybir.ActivationFunctionType.Sqrt, bias=eps_sb)
                nc.vector.reciprocal(out=mv[:, :, 3], in_=mv[:, :, 3])

                o_sb = resp.tile([128, NT, DIN], FP32, tag="o")
                for t in range(NT):
                    nc.vector.tensor_scalar(
                        out=o_sb[:, t, :], in0=res[:, t, :],
                        scalar1=mv[:, t, 0:1], scalar2=mv[:, t, 3:4],
                        op0=mybir.AluOpType.subtract,
                        op1=mybir.AluOpType.mult)
                if use_gb:
                    nc.gpsimd.tensor_mul(out=o_sb, in0=o_sb, in1=g_rep)
                    nc.gpsimd.tensor_add(out=o_sb, in0=o_sb, in1=b_rep)
                nc.sync.dma_start(
                    out=y_d[b].rearrange("(t p) j -> p t j", p=128), in_=o_sb)

    split_multiwaits(nc)
    return nc


def kernel(featureVec, Wqkv, Wo, bo, ln_gamma, ln_beta):
    x = np.ascontiguousarray(np.asarray(featureVec, dtype=np.float32))
    Wqkv = np.asarray(Wqkv, dtype=np.float32)
    Wo = np.asarray(Wo, dtype=np.float32)
    bo = np.asarray(bo, dtype=np.float32)
    g = np.asarray(ln_gamma, dtype=np.float32)
    be = np.asarray(ln_beta, dtype=np.float32)

    # host-side weight packing / folding
    wq_pack = np.concatenate([Wqkv[h, 0] * 0.125 for h in range(H)], axis=1)
    wk_pack = np.concatenate([Wqkv[h, 1] for h in range(H)], axis=1)
    wv_pack = np.concatenate(
        [(Wqkv[h, 2].astype(np.float64)
          @ Wo[h * DOUT:(h + 1) * DOUT].astype(np.float64)).astype(np.float32)
         for h in range(H)], axis=1)
    import ml_dtypes
    bf = ml_dtypes.bfloat16
    wq_host = np.ascontiguousarray(
        np.concatenate([wq_pack[:, 0:128], wq_pack[:, 128:256]],
                       axis=0).astype(bf))
    wk_host = np.ascontiguousarray(
        np.concatenate([wk_pack[:, 0:128], wk_pack[:, 128:256]],
                       axis=0).astype(bf))
    wv_host = np.ascontiguousarray(
        np.concatenate([wv_pack, wv_pack], axis=0).astype(bf))

    use_gb = not (np.all(g == 1.0) and np.all(be == 0.0))
    use_bo = not np.all(bo == 0.0)

    key = (use_gb, use_bo)
    if key not in _cache:
        _cache[key] = _build(use_gb, use_bo)
    nc = _cache[key]

    in_maps = []
    for c in range(NCORES):
        m = {
            "x": np.ascontiguousarray(x[c * BPC:(c + 1) * BPC]),
            "wq": wq_host, "wk": wk_host, "wv": wv_host,
        }
        if use_gb:
            m["gb"] = np.ascontiguousarray(np.stack([g, be]))
        if use_bo:
            m["bo"] = bo
        in_maps.append(m)

    res = run_bass_kernel_spmd(nc, in_maps, core_ids=list(range(NCORES)))
    return np.concatenate([r["y"] for r in res.results], axis=0)


if __name__ == "__main__":
    rng = np.random.default_rng(0)
    inputs = {
        "featureVec": rng.standard_normal((B, F, DIN), dtype=np.float32),
        "Wqkv": (rng.standard_normal((H, 3, DIN, DOUT), dtype=np.float32)
                 / np.sqrt(DIN).astype(np.float32)),
        "Wo": (rng.standard_normal((H * DOUT, DIN), dtype=np.float32)
               / np.sqrt(H * DOUT).astype(np.float32)),
        "bo": np.zeros(DIN, np.float32),
        "ln_gamma": np.ones(DIN, np.float32),
        "ln_beta": np.zeros(DIN, np.float32),
    }
    out = kernel(**inputs)
    print(out.shape, out.dtype, float(np.abs(out).max()))


# revision 22
# speedup vs baseline: 1.0938x; 1.0938x over previous
"""Trainium2 Bass kernel for a multi-head ReLU-attention transformer layer.

Shapes (hardcoded): B=32, F=1024, DIN=64, DOUT=64, H=4.
  qkv   = einsum("bfi,hkio->bhkfo", x, Wqkv)
  scores= relu(q @ k^T / sqrt(DOUT))
  head  = scores @ v
  out   = LN(concat(head) @ Wo + bo + x) * gamma + beta

Sharding: pure data-parallel over batch B across 8 NeuronCores (4 b/core).

Host-side algebraic folds (exact or fp32-precise):
  - 1/sqrt(DOUT)=0.125 folded into Wq (exact, power of two).
  - Wo folded into Wv:  proj = sum_h scores_h @ (Wv_h @ Wo_h).

Per-batch device pipeline (all matmuls bf16 with fp32 PSUM accumulation —
fp32/fp32r matmuls silently return zeros on this toolchain):
  x -> (bf16 cast, DMA-xbar transpose) xT, duplicated onto both partition
  halves so 64-deep contractions pack two-per-MM via PE row groups.
  Q^T/K^T per head-pair land stacked on partition halves; scoresT =
  relu(K^T_tile^T @ Q^T) drains PSUM->SBUF via ScalarE/VectorE (the
  bandwidth-critical path: PSUM fp32 reads are capped at 1 elem/lane/cycle);
  projT accumulates over heads and g-tiles into two [64,512] PSUM banks
  (matmul PSUM outputs must be bank-aligned on this hardware); DMA-xbar
  transposes back to natural layout; residual + LayerNorm in fp32; DMA out.

This walrus build accepts only ONE sync wait per instruction; Tile emits
multi-waits, so split_multiwaits() hoists extras onto NoOps post-schedule.
"""

import numpy as np

import concourse.bass as bass
import concourse.mybir as mybir
import concourse.tile as tile
from concourse.bass_utils import run_bass_kernel_spmd


def split_multiwaits(nc):
    """Hoist all but the last sync wait of any instruction onto standalone
    NoOps inserted just before it on the same engine — semantically identical
    (same-engine program order runs the waits first), but keeps every
    instruction within this walrus build's one-wait limit."""
    n_split = 0
    max_upd = 0

    def fix_block(bl):
        nonlocal n_split, max_upd
        insts = list(bl.instructions)
        out = []
        changed = False
        for inst in insts:
            si = inst.sync_info
            if si is not None:
                max_upd = max(max_upd, len(si.on_update))
                waits = list(si.on_wait)
                if len(waits) > 1:
                    for k, w in enumerate(waits[:-1]):
                        nop = mybir.InstNoOp(
                            name=f"{inst.name}-wsplit{k}", ins=[], outs=[])
                        nop.engine = inst.engine
                        nop.sync_info = mybir.SyncInfo(
                            on_wait=[w], on_update=[])
                        out.append(nop)
                    inst.sync_info = mybir.SyncInfo(
                        on_wait=[waits[-1]], on_update=list(si.on_update))
                    n_split += 1
                    changed = True
            out.append(inst)
        if changed:
            bl.instructions = out
        for sub in getattr(bl, "blocks", None) or []:
            fix_block(sub)

    for f in nc.m.functions:
        for bl in f.blocks:
            fix_block(bl)
    assert max_upd <= 1, f"need update-splitting too: {max_upd}"
    return n_split


B, F, DIN, DOUT, H = 32, 1024, 64, 64, 4
NCORES = 8
BPC = B // NCORES  # batches per core
NT = F // 128  # 8 f-tiles per batch
FP32 = mybir.dt.float32
BF16 = mybir.dt.bfloat16
EPS = 1e-5

_cache = {}


def _build(use_gb: bool, use_bo: bool, stage: int = 99):
    nc = bass.Bass("TRN2", target_bir_lowering=False, debug=False,
                   num_devices=NCORES)
    x_d = nc.dram_tensor("x", [BPC, F, DIN], FP32, kind="ExternalInput").ap()
    wq_d = nc.dram_tensor("wq", [128, 128], BF16, kind="ExternalInput").ap()
    wk_d = nc.dram_tensor("wk", [128, 128], BF16, kind="ExternalInput").ap()
    wv_d = nc.dram_tensor("wv", [128, 256], BF16, kind="ExternalInput").ap()
    if use_gb:
        gb_d = nc.dram_tensor("gb", [2, DIN], FP32, kind="ExternalInput").ap()
    if use_bo:
        bo_d = nc.dram_tensor("bo", [DIN], FP32, kind="ExternalInput").ap()
    y_d = nc.dram_tensor("y", [BPC, F, DIN], FP32, kind="ExternalOutput").ap()

    # alternate score drains between ScalarE and VectorE, weighted toward
    # ScalarE (1.2 GHz vs 0.96 GHz): ACT gets 6 of every 11
    drain_pat = [True, False, True, False, True, False, True, False,
                 True, False, True]
    drain_i = [0]

    def drain_relu(out_ap, in_ap):
        use_act = drain_pat[drain_i[0] % len(drain_pat)]
        drain_i[0] += 1
        if use_act:
            nc.scalar.activation(out=out_ap, in_=in_ap,
                                 func=mybir.ActivationFunctionType.Relu)
        else:
            nc.vector.tensor_scalar_max(out=out_ap, in0=in_ap, scalar1=0.0)

    def drain_copy(out_ap, in_ap, act):
        if act:
            nc.scalar.activation(out=out_ap, in_=in_ap,
                                 func=mybir.ActivationFunctionType.Copy)
        else:
            nc.vector.tensor_copy(out=out_ap, in_=in_ap)

    with tile.TileContext(nc) as tc:
        with (
            tc.tile_pool(name="const", bufs=1) as constp,
            tc.tile_pool(name="xp", bufs=3) as xp,
            tc.tile_pool(name="xtp", bufs=3) as xtp,
            tc.tile_pool(name="qkp", bufs=3) as qkp,
            tc.tile_pool(name="vp", bufs=3) as vp,
            tc.tile_pool(name="scp", bufs=8) as scp,
            tc.tile_pool(name="pjp", bufs=2) as pjp,
            tc.tile_pool(name="resp", bufs=2) as resp,
            tc.tile_pool(name="statp", bufs=4) as statp,
            tc.tile_pool(name="mm", bufs=6, space="PSUM") as psmm,
            tc.tile_pool(name="acc", bufs=2, space="PSUM") as psacc,
        ):
            # ---- constants ----
            eps_sb = constp.tile([128, 1], FP32)
            nc.vector.memset(eps_sb, EPS)
            wq_sb = constp.tile([128, 128], BF16)
            nc.sync.dma_start(out=wq_sb, in_=wq_d)
            wk_sb = constp.tile([128, 128], BF16)
            nc.sync.dma_start(out=wk_sb, in_=wk_d)
            wv_sb = constp.tile([128, 256], BF16)
            nc.sync.dma_start(out=wv_sb, in_=wv_d)
            if use_gb:
                g_rep = constp.tile([128, NT, DIN], FP32)
                b_rep = constp.tile([128, NT, DIN], FP32)
                for t in range(NT):
                    nc.gpsimd.dma_start(
                        out=g_rep[:, t, :],
                        in_=bass.AP(gb_d.tensor, 0, [[0, 128], [1, DIN]]))
                    nc.gpsimd.dma_start(
                        out=b_rep[:, t, :],
                        in_=bass.AP(gb_d.tensor, DIN, [[0, 128], [1, DIN]]))
            if use_bo:
                bo_rep = constp.tile([128, DIN], FP32)
                nc.gpsimd.dma_start(
                    out=bo_rep,
                    in_=bass.AP(bo_d.tensor, 0, [[0, 128], [1, DIN]]))

            for b in range(BPC):
                # ---- load x (natural: partition = f within tile) ----
                x_sb = xp.tile([128, NT, DIN], FP32, tag="x")
                nc.sync.dma_start(
                    out=x_sb, in_=x_d[b].rearrange("(t p) j -> p t j", p=128))
                if use_bo:
                    x_res = xp.tile([128, NT, DIN], FP32, tag="xres")
                    for t in range(NT):
                        nc.vector.tensor_add(
                            out=x_res[:, t, :], in0=x_sb[:, t, :], in1=bo_rep)
                else:
                    x_res = x_sb
                x_bf = xp.tile([128, NT, DIN], BF16, tag="xbf")
                nc.vector.tensor_copy(out=x_bf, in_=x_sb)

                # ---- transpose x -> xT [64, 1024] via DMA xbar, dup ----
                # xbar tiles are 16x128, so transpose f-tile PAIRS as
                # [128,128] blocks: top half = xT of even tile, bottom = odd.
                xt = xtp.tile([128, F], BF16, tag="xt")
                for u in range(NT // 2):
                    tmp = xtp.tile([128, 128], BF16, tag="tmpt",
                                   name=f"tmp{u}_{b}")
                    nc.sync.dma_start_transpose(
                        out=tmp,
                        in_=x_bf[:, 2 * u:2 * u + 2, :].rearrange(
                            "p t j -> p (t j)"))
                    nc.sync.dma_start(
                        out=xt[0:64, bass.ts(2 * u, 128)], in_=tmp[0:64, :])
                    nc.sync.dma_start(
                        out=xt[0:64, bass.ts(2 * u + 1, 128)],
                        in_=tmp[64:128, :])
                nc.sync.dma_start(out=xt[64:128, :], in_=xt[0:64, :])

                if stage < 2:
                    nc.sync.dma_start(
                        out=y_d[b].rearrange("(t p) j -> p t j", p=128),
                        in_=x_sb)
                    continue
                # ---- QKV projections (row-packed pairs) ----
                qk_sb = []
                for w_sb, nm in ((wq_sb, "q"), (wk_sb, "k")):
                    sb_a = qkp.tile([128, F], BF16, tag=nm + "a")
                    sb_b = qkp.tile([128, F], BF16, tag=nm + "b")
                    for fc in range(2):
                        fsl = bass.ts(fc, 512)
                        ps_a = psmm.tile([128, 512], FP32, tag="mm",
                                         name=f"qk_a_{nm}{fc}_{b}")
                        ps_b = psmm.tile([128, 512], FP32, tag="mm",
                                         name=f"qk_b_{nm}{fc}_{b}")
                        nc.tensor.matmul(
                            ps_a, w_sb[0:64, :],
                            xt[0:64, fsl], start=True, stop=True)
                        nc.tensor.matmul(
                            ps_b, w_sb[64:128, :],
                            xt[64:128, fsl], start=True, stop=True)
                        drain_copy(sb_a[:, fsl], ps_a, act=True)
                        drain_copy(sb_b[:, fsl], ps_b, act=False)
                    qk_sb.append((sb_a, sb_b))
                (qt_a, qt_b), (kt_a, kt_b) = qk_sb

                if stage < 3:
                    nc.sync.dma_start(
                        out=y_d[b].rearrange("(t p) j -> p t j", p=128),
                        in_=x_sb)
                    continue
                # v' = x @ (Wv@Wo): natural [g, (h o)=256], g-tile pairs
                # packed via row groups; one MM per PSUM bank (bank-aligned)
                vt = vp.tile([128, NT, 320], BF16, tag="v")
                nc.gpsimd.memset(vt[:, :, 256:320], 0.0)
                for gt in range(NT):
                    v_ps = psmm.tile([128, 512], FP32, tag="mm",
                                     name=f"v_ps{gt}_{b}")
                    half = gt % 2
                    nc.tensor.matmul(
                        v_ps[:, 0:256],
                        xt[bass.ds(64 * half, 64), bass.ts(gt, 128)],
                        wv_sb[bass.ds(64 * half, 64), :],
                        start=True, stop=True)
                    drain_copy(vt[:, gt, 0:256], v_ps[:, 0:256],
                               act=(gt % 2 == 0))

                if stage < 4:
                    nc.sync.dma_start(
                        out=y_d[b].rearrange("(t p) j -> p t j", p=128),
                        in_=x_sb)
                    continue
                # ---- attention: scoresT then projT accumulation ----
                # projT f-chunk accumulators [128, 512]: rows 0-63 hold the
                # real sum_h V'_h^T @ scT_h; rows 64-127 accumulate a
                # harmless byproduct of the M=128 head-pack (a matmul costs
                # N cycles regardless of M, so packing [V'_h|V'_h+1] into the
                # stationary operand halves the MM count vs M=64).
                out_f = [psacc.tile([128, 512], FP32, tag="acc",
                                    name=f"out_f{fc}_{b}")
                         for fc in range(2)]

                def emit_out_mms(hp, gt, sc0, sc1, first, last):
                    for fc in range(2):
                        # rows 0-63 += V'_{2hp}^T @ scT_{2hp}
                        nc.tensor.matmul(
                            out_f[fc][:, :],
                            vt[:, gt, bass.ds(128 * hp, 128)],
                            sc0[:, fc, :],
                            start=first, stop=False,
                            skip_group_check=True)
                        # rows 0-63 += V'_{2hp+1}^T @ scT_{2hp+1}
                        # (shifted slice: [V'_h1 | V'_h2] or [V'_h3 | 0])
                        nc.tensor.matmul(
                            out_f[fc][:, :],
                            vt[:, gt, bass.ds(128 * hp + 64, 128)],
                            sc1[:, fc, :],
                            start=False, stop=last,
                            skip_group_check=True)

                # software pipeline: defer each gt's out-MMs one iteration so
                # the in-order PE never head-of-line blocks on a score drain
                pending = None
                for hp in range(2):
                    qt = qt_a if hp == 0 else qt_b
                    kt = kt_a if hp == 0 else kt_b
                    for gt in range(NT):
                        gsl = bass.ts(gt, 128)
                        sc0 = scp.tile([128, 2, 512], BF16, tag="sc")
                        sc1 = scp.tile([128, 2, 512], BF16, tag="sc")
                        for fc in range(2):
                            fsl = bass.ts(fc, 512)
                            p0 = psmm.tile([128, 512], FP32, tag="mm",
                                           name=f"s0_{b}_{hp}_{gt}_{fc}")
                            p1 = psmm.tile([128, 512], FP32, tag="mm",
                                           name=f"s1_{b}_{hp}_{gt}_{fc}")
                            nc.tensor.matmul(
                                p0, kt[0:64, gsl], qt[0:64, fsl],
                                start=True, stop=True)
                            nc.tensor.matmul(
                                p1, kt[64:128, gsl], qt[64:128, fsl],
                                start=True, stop=True)
                            drain_relu(sc0[:, fc, :], p0)
                            drain_relu(sc1[:, fc, :], p1)
                        if pending is not None:
                            emit_out_mms(*pending)
                        pending = (hp, gt, sc0, sc1,
                                   hp == 0 and gt == 0,
                                   hp == 1 and gt == NT - 1)
                emit_out_mms(*pending)

                if stage < 5:
                    nc.sync.dma_start(
                        out=y_d[b].rearrange("(t p) j -> p t j", p=128),
                        in_=x_sb)
                    continue
                # ---- projT -> natural + residual + LayerNorm ----
                pj = pjp.tile([64, 2, 512], BF16, tag="pj")
                drain_copy(pj[:, 0, :], out_f[0][0:64, :], act=False)
                drain_copy(pj[:, 1, :], out_f[1][0:64, :], act=True)
                nat_sb = resp.tile([128, NT, DIN], BF16, tag="natsb")
                for t in range(NT):
                    fc, tw = divmod(t, 4)
                    nc.sync.dma_start_transpose(
                        out=nat_sb[:, t, :], in_=pj[:, fc, bass.ts(tw, 128)])
                res = resp.tile([128, NT, DIN], FP32, tag="res")
                nc.vector.tensor_add(out=res, in0=nat_sb, in1=x_res)

                sq = resp.tile([128, NT, DIN], FP32, tag="sq")
                nc.gpsimd.tensor_mul(out=sq, in0=res, in1=res)
                stat = statp.tile([128, NT, 2], FP32, tag="stat")
                nc.vector.tensor_reduce(
                    out=stat[:, :, 0], in_=res,
                    axis=mybir.AxisListType.X, op=mybir.AluOpType.add)
                nc.vector.tensor_reduce(
                    out=stat[:, :, 1], in_=sq,
                    axis=mybir.AxisListType.X, op=mybir.AluOpType.add)
                mv = statp.tile([128, NT, 4], FP32, tag="mv")
                # mean, E[x^2]
                nc.vector.tensor_scalar_mul(
                    out=mv[:, :, 0], in0=stat[:, :, 0], scalar1=1.0 / DIN)
                nc.vector.tensor_scalar_mul(
                    out=mv[:, :, 1], in0=stat[:, :, 1], scalar1=1.0 / DIN)
                # var = E[x^2] - mean^2
                nc.vector.tensor_mul(
                    out=mv[:, :, 2], in0=mv[:, :, 0], in1=mv[:, :, 0])
                nc.vector.tensor_sub(
                    out=mv[:, :, 2], in0=mv[:, :, 1], in1=mv[:, :, 2])
                # rstd = 1/sqrt(var + eps)
                nc.scalar.activation(
                    out=mv[:, :, 3], in_=mv[:, :, 2],
                    func=mybir.ActivationFunctionType.Sqrt, bias=eps_sb)
                nc.vector.reciprocal(out=mv[:, :, 3], in_=mv[:, :, 3])

                o_sb = resp.tile([128, NT, DIN], FP32, tag="o")
                for t in range(NT):
                    nc.vector.tensor_scalar(
                        out=o_sb[:, t, :], in0=res[:, t, :],
                        scalar1=mv[:, t, 0:1], scalar2=mv[:, t, 3:4],
                        op0=mybir.AluOpType.subtract,
                        op1=mybir.AluOpType.mult)
                if use_gb:
                    nc.gpsimd.tensor_mul(out=o_sb, in0=o_sb, in1=g_rep)
                    nc.gpsimd.tensor_add(out=o_sb, in0=o_sb, in1=b_rep)
                nc.sync.dma_start(
                    out=y_d[b].rearrange("(t p) j -> p t j", p=128), in_=o_sb)

    split_multiwaits(nc)
    return nc


def kernel(featureVec, Wqkv, Wo, bo, ln_gamma, ln_beta):
    x = np.ascontiguousarray(np.asarray(featureVec, dtype=np.float32))
    Wqkv = np.asarray(Wqkv, dtype=np.float32)
    Wo = np.asarray(Wo, dtype=np.float32)
    bo = np.asarray(bo, dtype=np.float32)
    g = np.asarray(ln_gamma, dtype=np.float32)
    be = np.asarray(ln_beta, dtype=np.float32)

    # host-side weight packing / folding
    wq_pack = np.concatenate([Wqkv[h, 0] * 0.125 for h in range(H)], axis=1)
    wk_pack = np.concatenate([Wqkv[h, 1] for h in range(H)], axis=1)
    wv_pack = np.concatenate(
        [(Wqkv[h, 2].astype(np.float64)
          @ Wo[h * DOUT:(h + 1) * DOUT].astype(np.float64)).astype(np.float32)
         for h in range(H)], axis=1)
    import ml_dtypes
    bf = ml_dtypes.bfloat16
    wq_host = np.ascontiguousarray(
        np.concatenate([wq_pack[:, 0:128], wq_pack[:, 128:256]],
                       axis=0).astype(bf))
    wk_host = np.ascontiguousarray(
        np.concatenate([wk_pack[:, 0:128], wk_pack[:, 128:256]],
                       axis=0).astype(bf))
    wv_host = np.ascontiguousarray(
        np.concatenate([wv_pack, wv_pack], axis=0).astype(bf))

    use_gb = not (np.all(g == 1.0) and np.all(be == 0.0))
    use_bo = not np.all(bo == 0.0)

    key = (use_gb, use_bo)
    if key not in _cache:
        _cache[key] = _build(use_gb, use_bo)
    nc = _cache[key]

    in_maps = []
    for c in range(NCORES):
        m = {
            "x": np.ascontiguousarray(x[c * BPC:(c + 1) * BPC]),
            "wq": wq_host, "wk": wk_host, "wv": wv_host,
        }
        if use_gb:
            m["gb"] = np.ascontiguousarray(np.stack([g, be]))
        if use_bo:
            m["bo"] = bo
        in_maps.append(m)

    res = run_bass_kernel_spmd(nc, in_maps, core_ids=list(range(NCORES)))
    return np.concatenate([r["y"] for r in res.results], axis=0)


if __name__ == "__main__":
    rng = np.random.default_rng(0)
    inputs = {
        "featureVec": rng.standard_normal((B, F, DIN), dtype=np.float32),
        "Wqkv": (rng.standard_normal((H, 3, DIN, DOUT), dtype=np.float32)
                 / np.sqrt(DIN).astype(np.float32)),
        "Wo": (rng.standard_normal((H * DOUT, DIN), dtype=np.float32)
               / np.sqrt(H * DOUT).astype(np.float32)),
        "bo": np.zeros(DIN, np.float32),
        "ln_gamma": np.ones(DIN, np.float32),
        "ln_beta": np.zeros(DIN, np.float32),
    }
    out = kernel(**inputs)
    print(out.shape, out.dtype, float(np.abs(out).max()))


# revision 26
# speedup vs baseline: 1.0940x; 1.0002x over previous
"""Trainium2 Bass kernel for a multi-head ReLU-attention transformer layer.

Shapes (hardcoded): B=32, F=1024, DIN=64, DOUT=64, H=4.
  qkv   = einsum("bfi,hkio->bhkfo", x, Wqkv)
  scores= relu(q @ k^T / sqrt(DOUT))
  head  = scores @ v
  out   = LN(concat(head) @ Wo + bo + x) * gamma + beta

Sharding: pure data-parallel over batch B across 8 NeuronCores (4 b/core).

Host-side algebraic folds (exact or fp32-precise):
  - 1/sqrt(DOUT)=0.125 folded into Wq (exact, power of two).
  - Wo folded into Wv:  proj = sum_h scores_h @ (Wv_h @ Wo_h).

Per-batch device pipeline (all matmuls bf16 with fp32 PSUM accumulation —
fp32/fp32r matmuls silently return zeros on this toolchain):
  x -> (bf16 cast, DMA-xbar transpose) xT, duplicated onto both partition
  halves so 64-deep contractions pack two-per-MM via PE row groups.
  Q^T/K^T per head-pair land stacked on partition halves; scoresT =
  relu(K^T_tile^T @ Q^T) drains PSUM->SBUF via ScalarE/VectorE (the
  bandwidth-critical path: PSUM fp32 reads are capped at 1 elem/lane/cycle);
  projT accumulates over heads and g-tiles into two [64,512] PSUM banks
  (matmul PSUM outputs must be bank-aligned on this hardware); DMA-xbar
  transposes back to natural layout; residual + LayerNorm in fp32; DMA out.

This walrus build accepts only ONE sync wait per instruction; Tile emits
multi-waits, so split_multiwaits() hoists extras onto NoOps post-schedule.
"""

import numpy as np

import concourse.bass as bass
import concourse.mybir as mybir
import concourse.tile as tile
from concourse.bass_utils import run_bass_kernel_spmd


def split_multiwaits(nc):
    """Hoist all but the last sync wait of any instruction onto standalone
    NoOps inserted just before it on the same engine — semantically identical
    (same-engine program order runs the waits first), but keeps every
    instruction within this walrus build's one-wait limit."""
    n_split = 0
    max_upd = 0

    def fix_block(bl):
        nonlocal n_split, max_upd
        insts = list(bl.instructions)
        out = []
        changed = False
        for inst in insts:
            si = inst.sync_info
            if si is not None:
                max_upd = max(max_upd, len(si.on_update))
                waits = list(si.on_wait)
                if len(waits) > 1:
                    for k, w in enumerate(waits[:-1]):
                        nop = mybir.InstNoOp(
                            name=f"{inst.name}-wsplit{k}", ins=[], outs=[])
                        nop.engine = inst.engine
                        nop.sync_info = mybir.SyncInfo(
                            on_wait=[w], on_update=[])
                        out.append(nop)
                    inst.sync_info = mybir.SyncInfo(
                        on_wait=[waits[-1]], on_update=list(si.on_update))
                    n_split += 1
                    changed = True
            out.append(inst)
        if changed:
            bl.instructions = out
        for sub in getattr(bl, "blocks", None) or []:
            fix_block(sub)

    for f in nc.m.functions:
        for bl in f.blocks:
            fix_block(bl)
    assert max_upd <= 1, f"need update-splitting too: {max_upd}"
    return n_split


B, F, DIN, DOUT, H = 32, 1024, 64, 64, 4
NCORES = 8
BPC = B // NCORES  # batches per core
NT = F // 128  # 8 f-tiles per batch
FP32 = mybir.dt.float32
BF16 = mybir.dt.bfloat16
EPS = 1e-5

_cache = {}


def _build(use_gb: bool, use_bo: bool, stage: int = 99):
    nc = bass.Bass("TRN2", target_bir_lowering=False, debug=False,
                   num_devices=NCORES)
    x_d = nc.dram_tensor("x", [BPC, F, DIN], FP32, kind="ExternalInput").ap()
    wq_d = nc.dram_tensor("wq", [128, 128], BF16, kind="ExternalInput").ap()
    wk_d = nc.dram_tensor("wk", [128, 128], BF16, kind="ExternalInput").ap()
    wv_d = nc.dram_tensor("wv", [128, 256], BF16, kind="ExternalInput").ap()
    if use_gb:
        gb_d = nc.dram_tensor("gb", [2, DIN], FP32, kind="ExternalInput").ap()
    if use_bo:
        bo_d = nc.dram_tensor("bo", [DIN], FP32, kind="ExternalInput").ap()
    y_d = nc.dram_tensor("y", [BPC, F, DIN], FP32, kind="ExternalOutput").ap()

    # alternate score drains between ScalarE and VectorE, weighted toward
    # ScalarE (1.2 GHz vs 0.96 GHz): ACT gets 6 of every 11
    drain_pat = [True, False, True, False, True, False, True, False,
                 True, False, True]
    drain_i = [0]

    def drain_relu(out_ap, in_ap):
        use_act = drain_pat[drain_i[0] % len(drain_pat)]
        drain_i[0] += 1
        if use_act:
            nc.scalar.activation(out=out_ap, in_=in_ap,
                                 func=mybir.ActivationFunctionType.Relu)
        else:
            nc.vector.tensor_scalar_max(out=out_ap, in0=in_ap, scalar1=0.0)

    def drain_copy(out_ap, in_ap, act=None):
        if act is None:
            act = drain_pat[drain_i[0] % len(drain_pat)]
            drain_i[0] += 1
        if act:
            nc.scalar.activation(out=out_ap, in_=in_ap,
                                 func=mybir.ActivationFunctionType.Copy)
        else:
            nc.vector.tensor_copy(out=out_ap, in_=in_ap)

    with tile.TileContext(nc) as tc:
        with (
            tc.tile_pool(name="const", bufs=1) as constp,
            tc.tile_pool(name="xp", bufs=3) as xp,
            tc.tile_pool(name="xtp", bufs=3) as xtp,
            tc.tile_pool(name="qkp", bufs=3) as qkp,
            tc.tile_pool(name="vp", bufs=3) as vp,
            tc.tile_pool(name="scp", bufs=12) as scp,
            tc.tile_pool(name="pjp", bufs=3) as pjp,
            tc.tile_pool(name="resp", bufs=3) as resp,
            tc.tile_pool(name="statp", bufs=4) as statp,
            tc.tile_pool(name="mm", bufs=6, space="PSUM") as psmm,
            tc.tile_pool(name="acc", bufs=2, space="PSUM") as psacc,
        ):
            # ---- constants ----
            eps_sb = constp.tile([128, 1], FP32)
            nc.vector.memset(eps_sb, EPS)
            wq_sb = constp.tile([128, 128], BF16)
            nc.sync.dma_start(out=wq_sb, in_=wq_d)
            wk_sb = constp.tile([128, 128], BF16)
            nc.sync.dma_start(out=wk_sb, in_=wk_d)
            wv_sb = constp.tile([128, 256], BF16)
            nc.sync.dma_start(out=wv_sb, in_=wv_d)
            if use_gb:
                g_rep = constp.tile([128, NT, DIN], FP32)
                b_rep = constp.tile([128, NT, DIN], FP32)
                for t in range(NT):
                    nc.gpsimd.dma_start(
                        out=g_rep[:, t, :],
                        in_=bass.AP(gb_d.tensor, 0, [[0, 128], [1, DIN]]))
                    nc.gpsimd.dma_start(
                        out=b_rep[:, t, :],
                        in_=bass.AP(gb_d.tensor, DIN, [[0, 128], [1, DIN]]))
            if use_bo:
                bo_rep = constp.tile([128, DIN], FP32)
                nc.gpsimd.dma_start(
                    out=bo_rep,
                    in_=bass.AP(bo_d.tensor, 0, [[0, 128], [1, DIN]]))

            for b in range(BPC):
                # ---- load x (natural: partition = f within tile) ----
                x_sb = xp.tile([128, NT, DIN], FP32, tag="x")
                nc.sync.dma_start(
                    out=x_sb, in_=x_d[b].rearrange("(t p) j -> p t j", p=128))
                if use_bo:
                    x_res = xp.tile([128, NT, DIN], FP32, tag="xres")
                    for t in range(NT):
                        nc.vector.tensor_add(
                            out=x_res[:, t, :], in0=x_sb[:, t, :], in1=bo_rep)
                else:
                    x_res = x_sb
                x_bf = xp.tile([128, NT, DIN], BF16, tag="xbf")
                nc.vector.tensor_copy(out=x_bf, in_=x_sb)

                # ---- transpose x -> xT [64, 1024] via DMA xbar, dup ----
                # xbar tiles are 16x128, so transpose f-tile PAIRS as
                # [128,128] blocks: top half = xT of even tile, bottom = odd.
                xt = xtp.tile([128, F], BF16, tag="xt")
                for u in range(NT // 2):
                    tmp = xtp.tile([128, 128], BF16, tag="tmpt",
                                   name=f"tmp{u}_{b}")
                    nc.sync.dma_start_transpose(
                        out=tmp,
                        in_=x_bf[:, 2 * u:2 * u + 2, :].rearrange(
                            "p t j -> p (t j)"))
                    nc.sync.dma_start(
                        out=xt[0:64, bass.ts(2 * u, 128)], in_=tmp[0:64, :])
                    nc.sync.dma_start(
                        out=xt[0:64, bass.ts(2 * u + 1, 128)],
                        in_=tmp[64:128, :])
                nc.sync.dma_start(out=xt[64:128, :], in_=xt[0:64, :])

                if stage < 2:
                    nc.sync.dma_start(
                        out=y_d[b].rearrange("(t p) j -> p t j", p=128),
                        in_=x_sb)
                    continue
                # ---- QKV projections (row-packed pairs) ----
                qk_sb = []
                for w_sb, nm in ((wq_sb, "q"), (wk_sb, "k")):
                    sb_a = qkp.tile([128, F], BF16, tag=nm + "a")
                    sb_b = qkp.tile([128, F], BF16, tag=nm + "b")
                    for fc in range(2):
                        fsl = bass.ts(fc, 512)
                        ps_a = psmm.tile([128, 512], FP32, tag="mm",
                                         name=f"qk_a_{nm}{fc}_{b}")
                        ps_b = psmm.tile([128, 512], FP32, tag="mm",
                                         name=f"qk_b_{nm}{fc}_{b}")
                        nc.tensor.matmul(
                            ps_a, w_sb[0:64, :],
                            xt[0:64, fsl], start=True, stop=True)
                        nc.tensor.matmul(
                            ps_b, w_sb[64:128, :],
                            xt[64:128, fsl], start=True, stop=True)
                        drain_copy(sb_a[:, fsl], ps_a)
                        drain_copy(sb_b[:, fsl], ps_b)
                    qk_sb.append((sb_a, sb_b))
                (qt_a, qt_b), (kt_a, kt_b) = qk_sb

                if stage < 3:
                    nc.sync.dma_start(
                        out=y_d[b].rearrange("(t p) j -> p t j", p=128),
                        in_=x_sb)
                    continue
                # v' = x @ (Wv@Wo): natural [g, (h o)=256], g-tile pairs
                # packed via row groups; one MM per PSUM bank (bank-aligned)
                vt = vp.tile([128, NT, 320], BF16, tag="v")
                nc.gpsimd.memset(vt[:, :, 256:320], 0.0)
                for gt in range(NT):
                    v_ps = psmm.tile([128, 512], FP32, tag="mm",
                                     name=f"v_ps{gt}_{b}")
                    half = gt % 2
                    nc.tensor.matmul(
                        v_ps[:, 0:256],
                        xt[bass.ds(64 * half, 64), bass.ts(gt, 128)],
                        wv_sb[bass.ds(64 * half, 64), :],
                        start=True, stop=True)
                    drain_copy(vt[:, gt, 0:256], v_ps[:, 0:256])

                if stage < 4:
                    nc.sync.dma_start(
                        out=y_d[b].rearrange("(t p) j -> p t j", p=128),
                        in_=x_sb)
                    continue
                # ---- attention: scoresT then projT accumulation ----
                # projT f-chunk accumulators [128, 512]: rows 0-63 hold the
                # real sum_h V'_h^T @ scT_h; rows 64-127 accumulate a
                # harmless byproduct of the M=128 head-pack (a matmul costs
                # N cycles regardless of M, so packing [V'_h|V'_h+1] into the
                # stationary operand halves the MM count vs M=64).
                out_f = [psacc.tile([128, 512], FP32, tag="acc",
                                    name=f"out_f{fc}_{b}")
                         for fc in range(2)]

                def emit_out_mms(hp, gt, sc0, sc1, first, last):
                    for fc in range(2):
                        # rows 0-63 += V'_{2hp}^T @ scT_{2hp}
                        nc.tensor.matmul(
                            out_f[fc][:, :],
                            vt[:, gt, bass.ds(128 * hp, 128)],
                            sc0[:, fc, :],
                            start=first, stop=False,
                            skip_group_check=True)
                        # rows 0-63 += V'_{2hp+1}^T @ scT_{2hp+1}
                        # (shifted slice: [V'_h1 | V'_h2] or [V'_h3 | 0])
                        nc.tensor.matmul(
                            out_f[fc][:, :],
                            vt[:, gt, bass.ds(128 * hp + 64, 128)],
                            sc1[:, fc, :],
                            start=False, stop=last,
                            skip_group_check=True)

                # software pipeline: defer each gt's out-MMs one iteration so
                # the in-order PE never head-of-line blocks on a score drain
                pending = None
                for hp in range(2):
                    qt = qt_a if hp == 0 else qt_b
                    kt = kt_a if hp == 0 else kt_b
                    for gt in range(NT):
                        gsl = bass.ts(gt, 128)
                        sc0 = scp.tile([128, 2, 512], BF16, tag="sc")
                        sc1 = scp.tile([128, 2, 512], BF16, tag="sc")
                        for fc in range(2):
                            fsl = bass.ts(fc, 512)
                            p0 = psmm.tile([128, 512], FP32, tag="mm",
                                           name=f"s0_{b}_{hp}_{gt}_{fc}")
                            p1 = psmm.tile([128, 512], FP32, tag="mm",
                                           name=f"s1_{b}_{hp}_{gt}_{fc}")
                            nc.tensor.matmul(
                                p0, kt[0:64, gsl], qt[0:64, fsl],
                                start=True, stop=True)
                            nc.tensor.matmul(
                                p1, kt[64:128, gsl], qt[64:128, fsl],
                                start=True, stop=True)
                            drain_relu(sc0[:, fc, :], p0)
                            drain_relu(sc1[:, fc, :], p1)
                        if pending is not None:
                            emit_out_mms(*pending)
                        pending = (hp, gt, sc0, sc1,
                                   hp == 0 and gt == 0,
                                   hp == 1 and gt == NT - 1)
                emit_out_mms(*pending)

                if stage < 5:
                    nc.sync.dma_start(
                        out=y_d[b].rearrange("(t p) j -> p t j", p=128),
                        in_=x_sb)
                    continue
                # ---- projT -> natural + residual + LayerNorm ----
                pj = pjp.tile([64, 2, 512], BF16, tag="pj")
                drain_copy(pj[:, 0, :], out_f[0][0:64, :])
                drain_copy(pj[:, 1, :], out_f[1][0:64, :])
                nat_sb = resp.tile([128, NT, DIN], BF16, tag="natsb")
                for t in range(NT):
                    fc, tw = divmod(t, 4)
                    nc.sync.dma_start_transpose(
                        out=nat_sb[:, t, :], in_=pj[:, fc, bass.ts(tw, 128)])
                res = resp.tile([128, NT, DIN], FP32, tag="res")
                nc.vector.tensor_add(out=res, in0=nat_sb, in1=x_res)

                sq = resp.tile([128, NT, DIN], FP32, tag="sq")
                nc.gpsimd.tensor_mul(out=sq, in0=res, in1=res)
                stat = statp.tile([128, NT, 2], FP32, tag="stat")
                nc.vector.tensor_reduce(
                    out=stat[:, :, 0], in_=res,
                    axis=mybir.AxisListType.X, op=mybir.AluOpType.add)
                nc.vector.tensor_reduce(
                    out=stat[:, :, 1], in_=sq,
                    axis=mybir.AxisListType.X, op=mybir.AluOpType.add)
                mv = statp.tile([128, NT, 4], FP32, tag="mv")
                # mean, E[x^2]
                nc.vector.tensor_scalar_mul(
                    out=mv[:, :, 0], in0=stat[:, :, 0], scalar1=1.0 / DIN)
                nc.vector.tensor_scalar_mul(
                    out=mv[:, :, 1], in0=stat[:, :, 1], scalar1=1.0 / DIN)
                # var = E[x^2] - mean^2
                nc.vector.tensor_mul(
                    out=mv[:, :, 2], in0=mv[:, :, 0], in1=mv[:, :, 0])
                nc.vector.tensor_sub(
                    out=mv[:, :, 2], in0=mv[:, :, 1], in1=mv[:, :, 2])
                # rstd = 1/sqrt(var + eps)
                nc.scalar.activation(
                    out=mv[:, :, 3], in_=mv[:, :, 2],
                    func=mybir.ActivationFunctionType.Sqrt, bias=eps_sb)
                nc.vector.reciprocal(out=mv[:, :, 3], in_=mv[:, :, 3])

                o_sb = resp.tile([128, NT, DIN], FP32, tag="o")
                for t in range(NT):
                    nc.vector.tensor_scalar(
                        out=o_sb[:, t, :], in0=res[:, t, :],
                        scalar1=mv[:, t, 0:1], scalar2=mv[:, t, 3:4],
                        op0=mybir.AluOpType.subtract,
                        op1=mybir.AluOpType.mult)
                if use_gb:
                    nc.gpsimd.tensor_mul(out=o_sb, in0=o_sb, in1=g_rep)
                    nc.gpsimd.tensor_add(out=o_sb, in0=o_sb, in1=b_rep)
                nc.sync.dma_start(
                    out=y_d[b].rearrange("(t p) j -> p t j", p=128), in_=o_sb)

    split_multiwaits(nc)
    return nc


def kernel(featureVec, Wqkv, Wo, bo, ln_gamma, ln_beta):
    x = np.ascontiguousarray(np.asarray(featureVec, dtype=np.float32))
    Wqkv = np.asarray(Wqkv, dtype=np.float32)
    Wo = np.asarray(Wo, dtype=np.float32)
    bo = np.asarray(bo, dtype=np.float32)
    g = np.asarray(ln_gamma, dtype=np.float32)
    be = np.asarray(ln_beta, dtype=np.float32)

    # host-side weight packing / folding
    wq_pack = np.concatenate([Wqkv[h, 0] * 0.125 for h in range(H)], axis=1)
    wk_pack = np.concatenate([Wqkv[h, 1] for h in range(H)], axis=1)
    wv_pack = np.concatenate(
        [(Wqkv[h, 2].astype(np.float64)
          @ Wo[h * DOUT:(h + 1) * DOUT].astype(np.float64)).astype(np.float32)
         for h in range(H)], axis=1)
    import ml_dtypes
    bf = ml_dtypes.bfloat16
    wq_host = np.ascontiguousarray(
        np.concatenate([wq_pack[:, 0:128], wq_pack[:, 128:256]],
                       axis=0).astype(bf))
    wk_host = np.ascontiguousarray(
        np.concatenate([wk_pack[:, 0:128], wk_pack[:, 128:256]],
                       axis=0).astype(bf))
    wv_host = np.ascontiguousarray(
        np.concatenate([wv_pack, wv_pack], axis=0).astype(bf))

    use_gb = not (np.all(g == 1.0) and np.all(be == 0.0))
    use_bo = not np.all(bo == 0.0)

    key = (use_gb, use_bo)
    if key not in _cache:
        _cache[key] = _build(use_gb, use_bo)
    nc = _cache[key]

    in_maps = []
    for c in range(NCORES):
        m = {
            "x": np.ascontiguousarray(x[c * BPC:(c + 1) * BPC]),
            "wq": wq_host, "wk": wk_host, "wv": wv_host,
        }
        if use_gb:
            m["gb"] = np.ascontiguousarray(np.stack([g, be]))
        if use_bo:
            m["bo"] = bo
        in_maps.append(m)

    res = run_bass_kernel_spmd(nc, in_maps, core_ids=list(range(NCORES)))
    return np.concatenate([r["y"] for r in res.results], axis=0)


if __name__ == "__main__":
    rng = np.random.default_rng(0)
    inputs = {
        "featureVec": rng.standard_normal((B, F, DIN), dtype=np.float32),
        "Wqkv": (rng.standard_normal((H, 3, DIN, DOUT), dtype=np.float32)
                 / np.sqrt(DIN).astype(np.float32)),
        "Wo": (rng.standard_normal((H * DOUT, DIN), dtype=np.float32)
               / np.sqrt(H * DOUT).astype(np.float32)),
        "bo": np.zeros(DIN, np.float32),
        "ln_gamma": np.ones(DIN, np.float32),
        "ln_beta": np.zeros(DIN, np.float32),
    }
    out = kernel(**inputs)
    print(out.shape, out.dtype, float(np.abs(out).max()))


# revision 28
# speedup vs baseline: 1.0943x; 1.0003x over previous
"""Trainium2 Bass kernel for a multi-head ReLU-attention transformer layer.

Shapes (hardcoded): B=32, F=1024, DIN=64, DOUT=64, H=4.
  qkv   = einsum("bfi,hkio->bhkfo", x, Wqkv)
  scores= relu(q @ k^T / sqrt(DOUT))
  head  = scores @ v
  out   = LN(concat(head) @ Wo + bo + x) * gamma + beta

Sharding: pure data-parallel over batch B across 8 NeuronCores (4 b/core).

Host-side algebraic folds (exact or fp32-precise):
  - 1/sqrt(DOUT)=0.125 folded into Wq (exact, power of two).
  - Wo folded into Wv:  proj = sum_h scores_h @ (Wv_h @ Wo_h).

Per-batch device pipeline (all matmuls bf16 with fp32 PSUM accumulation —
fp32/fp32r matmuls silently return zeros on this toolchain):
  x -> (bf16 cast, DMA-xbar transpose) xT, duplicated onto both partition
  halves so 64-deep contractions pack two-per-MM via PE row groups.
  Q^T/K^T per head-pair land stacked on partition halves; scoresT =
  relu(K^T_tile^T @ Q^T) drains PSUM->SBUF via ScalarE/VectorE (the
  bandwidth-critical path: PSUM fp32 reads are capped at 1 elem/lane/cycle);
  projT accumulates over heads and g-tiles into two [64,512] PSUM banks
  (matmul PSUM outputs must be bank-aligned on this hardware); DMA-xbar
  transposes back to natural layout; residual + LayerNorm in fp32; DMA out.

This walrus build accepts only ONE sync wait per instruction; Tile emits
multi-waits, so split_multiwaits() hoists extras onto NoOps post-schedule.
"""

import numpy as np

import concourse.bass as bass
import concourse.mybir as mybir
import concourse.tile as tile
from concourse.bass_utils import run_bass_kernel_spmd


def split_multiwaits(nc):
    """Hoist all but the last sync wait of any instruction onto standalone
    NoOps inserted just before it on the same engine — semantically identical
    (same-engine program order runs the waits first), but keeps every
    instruction within this walrus build's one-wait limit."""
    n_split = 0
    max_upd = 0

    def fix_block(bl):
        nonlocal n_split, max_upd
        insts = list(bl.instructions)
        out = []
        changed = False
        for inst in insts:
            si = inst.sync_info
            if si is not None:
                max_upd = max(max_upd, len(si.on_update))
                waits = list(si.on_wait)
                if len(waits) > 1:
                    for k, w in enumerate(waits[:-1]):
                        nop = mybir.InstNoOp(
                            name=f"{inst.name}-wsplit{k}", ins=[], outs=[])
                        nop.engine = inst.engine
                        nop.sync_info = mybir.SyncInfo(
                            on_wait=[w], on_update=[])
                        out.append(nop)
                    inst.sync_info = mybir.SyncInfo(
                        on_wait=[waits[-1]], on_update=list(si.on_update))
                    n_split += 1
                    changed = True
            out.append(inst)
        if changed:
            bl.instructions = out
        for sub in getattr(bl, "blocks", None) or []:
            fix_block(sub)

    for f in nc.m.functions:
        for bl in f.blocks:
            fix_block(bl)
    assert max_upd <= 1, f"need update-splitting too: {max_upd}"
    return n_split


B, F, DIN, DOUT, H = 32, 1024, 64, 64, 4
NCORES = 8
BPC = B // NCORES  # batches per core
NT = F // 128  # 8 f-tiles per batch
FP32 = mybir.dt.float32
BF16 = mybir.dt.bfloat16
EPS = 1e-5

_cache = {}


def _build(use_gb: bool, use_bo: bool, stage: int = 99):
    nc = bass.Bass("TRN2", target_bir_lowering=False, debug=False,
                   num_devices=NCORES)
    x_d = nc.dram_tensor("x", [BPC, F, DIN], FP32, kind="ExternalInput").ap()
    wq_d = nc.dram_tensor("wq", [128, 128], BF16, kind="ExternalInput").ap()
    wk_d = nc.dram_tensor("wk", [128, 128], BF16, kind="ExternalInput").ap()
    wv_d = nc.dram_tensor("wv", [128, 256], BF16, kind="ExternalInput").ap()
    if use_gb:
        gb_d = nc.dram_tensor("gb", [2, DIN], FP32, kind="ExternalInput").ap()
    if use_bo:
        bo_d = nc.dram_tensor("bo", [DIN], FP32, kind="ExternalInput").ap()
    y_d = nc.dram_tensor("y", [BPC, F, DIN], FP32, kind="ExternalOutput").ap()

    # alternate score drains between ScalarE and VectorE, weighted toward
    # ScalarE (1.2 GHz vs 0.96 GHz): ACT gets 8 of every 15 (equal-time
    # balance point measured via the cost model)
    drain_pat = ([True, False] * 7 + [True])
    drain_i = [0]

    def drain_relu(out_ap, in_ap):
        use_act = drain_pat[drain_i[0] % len(drain_pat)]
        drain_i[0] += 1
        if use_act:
            nc.scalar.activation(out=out_ap, in_=in_ap,
                                 func=mybir.ActivationFunctionType.Relu)
        else:
            nc.vector.tensor_scalar_max(out=out_ap, in0=in_ap, scalar1=0.0)

    def drain_copy(out_ap, in_ap, act=None):
        if act is None:
            act = drain_pat[drain_i[0] % len(drain_pat)]
            drain_i[0] += 1
        if act:
            nc.scalar.activation(out=out_ap, in_=in_ap,
                                 func=mybir.ActivationFunctionType.Copy)
        else:
            nc.vector.tensor_copy(out=out_ap, in_=in_ap)

    with tile.TileContext(nc) as tc:
        with (
            tc.tile_pool(name="const", bufs=1) as constp,
            tc.tile_pool(name="xp", bufs=3) as xp,
            tc.tile_pool(name="xtp", bufs=3) as xtp,
            tc.tile_pool(name="qkp", bufs=3) as qkp,
            tc.tile_pool(name="vp", bufs=3) as vp,
            tc.tile_pool(name="scp", bufs=12) as scp,
            tc.tile_pool(name="pjp", bufs=3) as pjp,
            tc.tile_pool(name="resp", bufs=3) as resp,
            tc.tile_pool(name="statp", bufs=4) as statp,
            tc.tile_pool(name="mm", bufs=6, space="PSUM") as psmm,
            tc.tile_pool(name="acc", bufs=2, space="PSUM") as psacc,
        ):
            # ---- constants ----
            eps_sb = constp.tile([128, 1], FP32)
            nc.vector.memset(eps_sb, EPS)
            wq_sb = constp.tile([128, 128], BF16)
            nc.sync.dma_start(out=wq_sb, in_=wq_d)
            wk_sb = constp.tile([128, 128], BF16)
            nc.sync.dma_start(out=wk_sb, in_=wk_d)
            wv_sb = constp.tile([128, 256], BF16)
            nc.sync.dma_start(out=wv_sb, in_=wv_d)
            if use_gb:
                g_rep = constp.tile([128, NT, DIN], FP32)
                b_rep = constp.tile([128, NT, DIN], FP32)
                for t in range(NT):
                    nc.gpsimd.dma_start(
                        out=g_rep[:, t, :],
                        in_=bass.AP(gb_d.tensor, 0, [[0, 128], [1, DIN]]))
                    nc.gpsimd.dma_start(
                        out=b_rep[:, t, :],
                        in_=bass.AP(gb_d.tensor, DIN, [[0, 128], [1, DIN]]))
            if use_bo:
                bo_rep = constp.tile([128, DIN], FP32)
                nc.gpsimd.dma_start(
                    out=bo_rep,
                    in_=bass.AP(bo_d.tensor, 0, [[0, 128], [1, DIN]]))

            for b in range(BPC):
                # ---- load x (natural: partition = f within tile) ----
                x_sb = xp.tile([128, NT, DIN], FP32, tag="x")
                nc.sync.dma_start(
                    out=x_sb, in_=x_d[b].rearrange("(t p) j -> p t j", p=128))
                if use_bo:
                    x_res = xp.tile([128, NT, DIN], FP32, tag="xres")
                    for t in range(NT):
                        nc.vector.tensor_add(
                            out=x_res[:, t, :], in0=x_sb[:, t, :], in1=bo_rep)
                else:
                    x_res = x_sb
                x_bf = xp.tile([128, NT, DIN], BF16, tag="xbf")
                nc.vector.tensor_copy(out=x_bf, in_=x_sb)

                # ---- transpose x -> xT [64, 1024] via DMA xbar, dup ----
                # xbar tiles are 16x128, so transpose f-tile PAIRS as
                # [128,128] blocks: top half = xT of even tile, bottom = odd.
                xt = xtp.tile([128, F], BF16, tag="xt")
                for u in range(NT // 2):
                    tmp = xtp.tile([128, 128], BF16, tag="tmpt",
                                   name=f"tmp{u}_{b}")
                    nc.sync.dma_start_transpose(
                        out=tmp,
                        in_=x_bf[:, 2 * u:2 * u + 2, :].rearrange(
                            "p t j -> p (t j)"))
                    nc.sync.dma_start(
                        out=xt[0:64, bass.ts(2 * u, 128)], in_=tmp[0:64, :])
                    nc.sync.dma_start(
                        out=xt[0:64, bass.ts(2 * u + 1, 128)],
                        in_=tmp[64:128, :])
                nc.sync.dma_start(out=xt[64:128, :], in_=xt[0:64, :])

                if stage < 2:
                    nc.sync.dma_start(
                        out=y_d[b].rearrange("(t p) j -> p t j", p=128),
                        in_=x_sb)
                    continue
                # ---- QKV projections (row-packed pairs) ----
                qk_sb = []
                for w_sb, nm in ((wq_sb, "q"), (wk_sb, "k")):
                    sb_a = qkp.tile([128, F], BF16, tag=nm + "a")
                    sb_b = qkp.tile([128, F], BF16, tag=nm + "b")
                    for fc in range(2):
                        fsl = bass.ts(fc, 512)
                        ps_a = psmm.tile([128, 512], FP32, tag="mm",
                                         name=f"qk_a_{nm}{fc}_{b}")
                        ps_b = psmm.tile([128, 512], FP32, tag="mm",
                                         name=f"qk_b_{nm}{fc}_{b}")
                        nc.tensor.matmul(
                            ps_a, w_sb[0:64, :],
                            xt[0:64, fsl], start=True, stop=True)
                        nc.tensor.matmul(
                            ps_b, w_sb[64:128, :],
                            xt[64:128, fsl], start=True, stop=True)
                        drain_copy(sb_a[:, fsl], ps_a)
                        drain_copy(sb_b[:, fsl], ps_b)
                    qk_sb.append((sb_a, sb_b))
                (qt_a, qt_b), (kt_a, kt_b) = qk_sb

                if stage < 3:
                    nc.sync.dma_start(
                        out=y_d[b].rearrange("(t p) j -> p t j", p=128),
                        in_=x_sb)
                    continue
                # v' = x @ (Wv@Wo): natural [g, (h o)=256], g-tile pairs
                # packed via row groups; one MM per PSUM bank (bank-aligned)
                vt = vp.tile([128, NT, 320], BF16, tag="v")
                nc.gpsimd.memset(vt[:, :, 256:320], 0.0)
                for gt in range(NT):
                    v_ps = psmm.tile([128, 512], FP32, tag="mm",
                                     name=f"v_ps{gt}_{b}")
                    half = gt % 2
                    nc.tensor.matmul(
                        v_ps[:, 0:256],
                        xt[bass.ds(64 * half, 64), bass.ts(gt, 128)],
                        wv_sb[bass.ds(64 * half, 64), :],
                        start=True, stop=True)
                    drain_copy(vt[:, gt, 0:256], v_ps[:, 0:256])

                if stage < 4:
                    nc.sync.dma_start(
                        out=y_d[b].rearrange("(t p) j -> p t j", p=128),
                        in_=x_sb)
                    continue
                # ---- attention: scoresT then projT accumulation ----
                # projT f-chunk accumulators [128, 512]: rows 0-63 hold the
                # real sum_h V'_h^T @ scT_h; rows 64-127 accumulate a
                # harmless byproduct of the M=128 head-pack (a matmul costs
                # N cycles regardless of M, so packing [V'_h|V'_h+1] into the
                # stationary operand halves the MM count vs M=64).
                out_f = [psacc.tile([128, 512], FP32, tag="acc",
                                    name=f"out_f{fc}_{b}")
                         for fc in range(2)]

                def emit_out_mms(hp, gt, sc0, sc1, first, last):
                    for fc in range(2):
                        # rows 0-63 += V'_{2hp}^T @ scT_{2hp}
                        nc.tensor.matmul(
                            out_f[fc][:, :],
                            vt[:, gt, bass.ds(128 * hp, 128)],
                            sc0[:, fc, :],
                            start=first, stop=False,
                            skip_group_check=True)
                        # rows 0-63 += V'_{2hp+1}^T @ scT_{2hp+1}
                        # (shifted slice: [V'_h1 | V'_h2] or [V'_h3 | 0])
                        nc.tensor.matmul(
                            out_f[fc][:, :],
                            vt[:, gt, bass.ds(128 * hp + 64, 128)],
                            sc1[:, fc, :],
                            start=False, stop=last,
                            skip_group_check=True)

                # software pipeline: defer each gt's out-MMs one iteration so
                # the in-order PE never head-of-line blocks on a score drain
                pending = None
                for hp in range(2):
                    qt = qt_a if hp == 0 else qt_b
                    kt = kt_a if hp == 0 else kt_b
                    for gt in range(NT):
                        gsl = bass.ts(gt, 128)
                        sc0 = scp.tile([128, 2, 512], BF16, tag="sc")
                        sc1 = scp.tile([128, 2, 512], BF16, tag="sc")
                        for fc in range(2):
                            fsl = bass.ts(fc, 512)
                            p0 = psmm.tile([128, 512], FP32, tag="mm",
                                           name=f"s0_{b}_{hp}_{gt}_{fc}")
                            p1 = psmm.tile([128, 512], FP32, tag="mm",
                                           name=f"s1_{b}_{hp}_{gt}_{fc}")
                            nc.tensor.matmul(
                                p0, kt[0:64, gsl], qt[0:64, fsl],
                                start=True, stop=True)
                            nc.tensor.matmul(
                                p1, kt[64:128, gsl], qt[64:128, fsl],
                                start=True, stop=True)
                            drain_relu(sc0[:, fc, :], p0)
                            drain_relu(sc1[:, fc, :], p1)
                        if pending is not None:
                            emit_out_mms(*pending)
                        pending = (hp, gt, sc0, sc1,
                                   hp == 0 and gt == 0,
                                   hp == 1 and gt == NT - 1)
                emit_out_mms(*pending)

                if stage < 5:
                    nc.sync.dma_start(
                        out=y_d[b].rearrange("(t p) j -> p t j", p=128),
                        in_=x_sb)
                    continue
                # ---- projT -> natural + residual + LayerNorm ----
                pj = pjp.tile([64, 2, 512], BF16, tag="pj")
                drain_copy(pj[:, 0, :], out_f[0][0:64, :])
                drain_copy(pj[:, 1, :], out_f[1][0:64, :])
                nat_sb = resp.tile([128, NT, DIN], BF16, tag="natsb")
                for t in range(NT):
                    fc, tw = divmod(t, 4)
                    nc.sync.dma_start_transpose(
                        out=nat_sb[:, t, :], in_=pj[:, fc, bass.ts(tw, 128)])
                res = resp.tile([128, NT, DIN], FP32, tag="res")
                nc.vector.tensor_add(out=res, in0=nat_sb, in1=x_res)

                sq = resp.tile([128, NT, DIN], FP32, tag="sq")
                nc.gpsimd.tensor_mul(out=sq, in0=res, in1=res)
                stat = statp.tile([128, NT, 2], FP32, tag="stat")
                nc.vector.tensor_reduce(
                    out=stat[:, :, 0], in_=res,
                    axis=mybir.AxisListType.X, op=mybir.AluOpType.add)
                nc.vector.tensor_reduce(
                    out=stat[:, :, 1], in_=sq,
                    axis=mybir.AxisListType.X, op=mybir.AluOpType.add)
                mv = statp.tile([128, NT, 4], FP32, tag="mv")
                # mean, E[x^2]
                nc.vector.tensor_scalar_mul(
                    out=mv[:, :, 0], in0=stat[:, :, 0], scalar1=1.0 / DIN)
                nc.vector.tensor_scalar_mul(
                    out=mv[:, :, 1], in0=stat[:, :, 1], scalar1=1.0 / DIN)
                # var = E[x^2] - mean^2
                nc.vector.tensor_mul(
                    out=mv[:, :, 2], in0=mv[:, :, 0], in1=mv[:, :, 0])
                nc.vector.tensor_sub(
                    out=mv[:, :, 2], in0=mv[:, :, 1], in1=mv[:, :, 2])
                # rstd = 1/sqrt(var + eps)
                nc.scalar.activation(
                    out=mv[:, :, 3], in_=mv[:, :, 2],
                    func=mybir.ActivationFunctionType.Sqrt, bias=eps_sb)
                nc.vector.reciprocal(out=mv[:, :, 3], in_=mv[:, :, 3])

                o_sb = resp.tile([128, NT, DIN], FP32, tag="o")
                for t in range(NT):
                    nc.vector.tensor_scalar(
                        out=o_sb[:, t, :], in0=res[:, t, :],
                        scalar1=mv[:, t, 0:1], scalar2=mv[:, t, 3:4],
                        op0=mybir.AluOpType.subtract,
                        op1=mybir.AluOpType.mult)
                if use_gb:
                    nc.gpsimd.tensor_mul(out=o_sb, in0=o_sb, in1=g_rep)
                    nc.gpsimd.tensor_add(out=o_sb, in0=o_sb, in1=b_rep)
                nc.sync.dma_start(
                    out=y_d[b].rearrange("(t p) j -> p t j", p=128), in_=o_sb)

    split_multiwaits(nc)
    return nc


def kernel(featureVec, Wqkv, Wo, bo, ln_gamma, ln_beta):
    x = np.ascontiguousarray(np.asarray(featureVec, dtype=np.float32))
    Wqkv = np.asarray(Wqkv, dtype=np.float32)
    Wo = np.asarray(Wo, dtype=np.float32)
    bo = np.asarray(bo, dtype=np.float32)
    g = np.asarray(ln_gamma, dtype=np.float32)
    be = np.asarray(ln_beta, dtype=np.float32)

    # host-side weight packing / folding
    wq_pack = np.concatenate([Wqkv[h, 0] * 0.125 for h in range(H)], axis=1)
    wk_pack = np.concatenate([Wqkv[h, 1] for h in range(H)], axis=1)
    wv_pack = np.concatenate(
        [(Wqkv[h, 2].astype(np.float64)
          @ Wo[h * DOUT:(h + 1) * DOUT].astype(np.float64)).astype(np.float32)
         for h in range(H)], axis=1)
    import ml_dtypes
    bf = ml_dtypes.bfloat16
    wq_host = np.ascontiguousarray(
        np.concatenate([wq_pack[:, 0:128], wq_pack[:, 128:256]],
                       axis=0).astype(bf))
    wk_host = np.ascontiguousarray(
        np.concatenate([wk_pack[:, 0:128], wk_pack[:, 128:256]],
                       axis=0).astype(bf))
    wv_host = np.ascontiguousarray(
        np.concatenate([wv_pack, wv_pack], axis=0).astype(bf))

    use_gb = not (np.all(g == 1.0) and np.all(be == 0.0))
    use_bo = not np.all(bo == 0.0)

    key = (use_gb, use_bo)
    if key not in _cache:
        _cache[key] = _build(use_gb, use_bo)
    nc = _cache[key]

    in_maps = []
    for c in range(NCORES):
        m = {
            "x": np.ascontiguousarray(x[c * BPC:(c + 1) * BPC]),
            "wq": wq_host, "wk": wk_host, "wv": wv_host,
        }
        if use_gb:
            m["gb"] = np.ascontiguousarray(np.stack([g, be]))
        if use_bo:
            m["bo"] = bo
        in_maps.append(m)

    res = run_bass_kernel_spmd(nc, in_maps, core_ids=list(range(NCORES)))
    return np.concatenate([r["y"] for r in res.results], axis=0)


if __name__ == "__main__":
    rng = np.random.default_rng(0)
    inputs = {
        "featureVec": rng.standard_normal((B, F, DIN), dtype=np.float32),
        "Wqkv": (rng.standard_normal((H, 3, DIN, DOUT), dtype=np.float32)
                 / np.sqrt(DIN).astype(np.float32)),
        "Wo": (rng.standard_normal((H * DOUT, DIN), dtype=np.float32)
               / np.sqrt(H * DOUT).astype(np.float32)),
        "bo": np.zeros(DIN, np.float32),
        "ln_gamma": np.ones(DIN, np.float32),
        "ln_beta": np.zeros(DIN, np.float32),
    }
    out = kernel(**inputs)
    print(out.shape, out.dtype, float(np.abs(out).max()))


# revision 33
# speedup vs baseline: 1.1053x; 1.0101x over previous
"""Trainium2 Bass kernel for a multi-head ReLU-attention transformer layer.

Shapes (hardcoded): B=32, F=1024, DIN=64, DOUT=64, H=4.
  qkv   = einsum("bfi,hkio->bhkfo", x, Wqkv)
  scores= relu(q @ k^T / sqrt(DOUT))
  head  = scores @ v
  out   = LN(concat(head) @ Wo + bo + x) * gamma + beta

Sharding: pure data-parallel over batch B across 8 NeuronCores (4 b/core).

Host-side algebraic folds (exact or fp32-precise):
  - 1/sqrt(DOUT)=0.125 folded into Wq (exact, power of two).
  - Wo folded into Wv:  proj = sum_h scores_h @ (Wv_h @ Wo_h).

Per-batch device pipeline (all matmuls bf16 with fp32 PSUM accumulation —
fp32/fp32r matmuls silently return zeros on this toolchain):
  x -> (bf16 cast, DMA-xbar transpose) xT, duplicated onto both partition
  halves so 64-deep contractions pack two-per-MM via PE row groups.
  Q^T/K^T per head-pair land stacked on partition halves; scoresT =
  relu(K^T_tile^T @ Q^T) drains PSUM->SBUF via ScalarE/VectorE (the
  bandwidth-critical path: PSUM fp32 reads are capped at 1 elem/lane/cycle);
  projT accumulates over heads and g-tiles into two [64,512] PSUM banks
  (matmul PSUM outputs must be bank-aligned on this hardware); DMA-xbar
  transposes back to natural layout; residual + LayerNorm in fp32; DMA out.

This walrus build accepts only ONE sync wait per instruction; Tile emits
multi-waits, so split_multiwaits() hoists extras onto NoOps post-schedule.
"""

import numpy as np

import concourse.bass as bass
import concourse.mybir as mybir
import concourse.tile as tile
from concourse.bass_utils import run_bass_kernel_spmd


def split_multiwaits(nc):
    """Hoist all but the last sync wait of any instruction onto standalone
    NoOps inserted just before it on the same engine — semantically identical
    (same-engine program order runs the waits first), but keeps every
    instruction within this walrus build's one-wait limit."""
    n_split = 0
    max_upd = 0

    def fix_block(bl):
        nonlocal n_split, max_upd
        insts = list(bl.instructions)
        out = []
        changed = False
        for inst in insts:
            si = inst.sync_info
            if si is not None:
                max_upd = max(max_upd, len(si.on_update))
                waits = list(si.on_wait)
                if len(waits) > 1:
                    for k, w in enumerate(waits[:-1]):
                        nop = mybir.InstNoOp(
                            name=f"{inst.name}-wsplit{k}", ins=[], outs=[])
                        nop.engine = inst.engine
                        nop.sync_info = mybir.SyncInfo(
                            on_wait=[w], on_update=[])
                        out.append(nop)
                    inst.sync_info = mybir.SyncInfo(
                        on_wait=[waits[-1]], on_update=list(si.on_update))
                    n_split += 1
                    changed = True
            out.append(inst)
        if changed:
            bl.instructions = out
        for sub in getattr(bl, "blocks", None) or []:
            fix_block(sub)

    for f in nc.m.functions:
        for bl in f.blocks:
            fix_block(bl)
    assert max_upd <= 1, f"need update-splitting too: {max_upd}"
    return n_split


B, F, DIN, DOUT, H = 32, 1024, 64, 64, 4
NCORES = 8
BPC = B // NCORES  # batches per core
NT = F // 128  # 8 f-tiles per batch
FP32 = mybir.dt.float32
BF16 = mybir.dt.bfloat16
EPS = 1e-5

_cache = {}


def _build(use_gb: bool, use_bo: bool, stage: int = 99):
    nc = bass.Bass("TRN2", target_bir_lowering=False, debug=False,
                   num_devices=NCORES)
    x_d = nc.dram_tensor("x", [BPC, F, DIN], FP32, kind="ExternalInput").ap()
    wq_d = nc.dram_tensor("wq", [128, 128], BF16, kind="ExternalInput").ap()
    wk_d = nc.dram_tensor("wk", [128, 128], BF16, kind="ExternalInput").ap()
    wv_d = nc.dram_tensor("wv", [128, 256], BF16, kind="ExternalInput").ap()
    if use_gb:
        gb_d = nc.dram_tensor("gb", [2, DIN], FP32, kind="ExternalInput").ap()
    if use_bo:
        bo_d = nc.dram_tensor("bo", [DIN], FP32, kind="ExternalInput").ap()
    y_d = nc.dram_tensor("y", [BPC, F, DIN], FP32, kind="ExternalOutput").ap()

    # alternate score drains between ScalarE and VectorE, weighted toward
    # ScalarE (1.2 GHz vs 0.96 GHz): ACT gets 8 of every 15 (equal-time
    # balance point measured via the cost model)
    drain_pat = ([True, False] * 7 + [True])
    drain_i = [0]

    def drain_relu(out_ap, in_ap):
        use_act = drain_pat[drain_i[0] % len(drain_pat)]
        drain_i[0] += 1
        if use_act:
            nc.scalar.activation(out=out_ap, in_=in_ap,
                                 func=mybir.ActivationFunctionType.Relu)
        else:
            nc.vector.tensor_scalar_max(out=out_ap, in0=in_ap, scalar1=0.0)

    def drain_copy(out_ap, in_ap, act=None):
        if act is None:
            act = drain_pat[drain_i[0] % len(drain_pat)]
            drain_i[0] += 1
        if act:
            nc.scalar.activation(out=out_ap, in_=in_ap,
                                 func=mybir.ActivationFunctionType.Copy)
        else:
            nc.vector.tensor_copy(out=out_ap, in_=in_ap)

    with tile.TileContext(nc) as tc:
        with (
            tc.tile_pool(name="const", bufs=1) as constp,
            tc.tile_pool(name="xp", bufs=3) as xp,
            tc.tile_pool(name="xtp", bufs=3) as xtp,
            tc.tile_pool(name="qkp", bufs=3) as qkp,
            tc.tile_pool(name="vp", bufs=3) as vp,
            tc.tile_pool(name="scp", bufs=12) as scp,
            tc.tile_pool(name="pjp", bufs=3) as pjp,
            tc.tile_pool(name="resp", bufs=3) as resp,
            tc.tile_pool(name="statp", bufs=4) as statp,
            tc.tile_pool(name="mm", bufs=6, space="PSUM") as psmm,
            tc.tile_pool(name="acc", bufs=2, space="PSUM") as psacc,
        ):
            # ---- constants ----
            eps_sb = constp.tile([128, 1], FP32)
            nc.vector.memset(eps_sb, EPS)
            wq_sb = constp.tile([128, 128], BF16)
            nc.sync.dma_start(out=wq_sb, in_=wq_d)
            wk_sb = constp.tile([128, 128], BF16)
            nc.sync.dma_start(out=wk_sb, in_=wk_d)
            wv_sb = constp.tile([128, 256], BF16)
            nc.sync.dma_start(out=wv_sb, in_=wv_d)
            if use_gb:
                g_rep = constp.tile([128, NT, DIN], FP32)
                b_rep = constp.tile([128, NT, DIN], FP32)
                for t in range(NT):
                    nc.gpsimd.dma_start(
                        out=g_rep[:, t, :],
                        in_=bass.AP(gb_d.tensor, 0, [[0, 128], [1, DIN]]))
                    nc.gpsimd.dma_start(
                        out=b_rep[:, t, :],
                        in_=bass.AP(gb_d.tensor, DIN, [[0, 128], [1, DIN]]))
            if use_bo:
                bo_rep = constp.tile([128, DIN], FP32)
                nc.gpsimd.dma_start(
                    out=bo_rep,
                    in_=bass.AP(bo_d.tensor, 0, [[0, 128], [1, DIN]]))

            for b in range(BPC):
                # ---- load x (natural: partition = f within tile) ----
                x_sb = xp.tile([128, NT, DIN], FP32, tag="x")
                nc.sync.dma_start(
                    out=x_sb, in_=x_d[b].rearrange("(t p) j -> p t j", p=128))
                if use_bo:
                    x_res = xp.tile([128, NT, DIN], FP32, tag="xres")
                    for t in range(NT):
                        nc.vector.tensor_add(
                            out=x_res[:, t, :], in0=x_sb[:, t, :], in1=bo_rep)
                else:
                    x_res = x_sb
                x_bf = xp.tile([128, NT, DIN], BF16, tag="xbf")
                nc.vector.tensor_copy(out=x_bf, in_=x_sb)

                # ---- transpose x -> xT [64, 1024] via DMA xbar, dup ----
                # xbar tiles are 16x128, so transpose f-tile PAIRS as
                # [128,128] blocks: top half = xT of even tile, bottom = odd.
                # All transposes issue before all copies: every
                # DMATranspose<->DMACopy xbar-mode transition serializes the
                # DMA path on this hardware, so batch the modes.
                xt = xtp.tile([128, F], BF16, tag="xt")
                tmp = xtp.tile([128, NT // 2, 128], BF16, tag="tmpt")
                for u in range(NT // 2):
                    nc.sync.dma_start_transpose(
                        out=tmp[:, u, :],
                        in_=x_bf[:, 2 * u:2 * u + 2, :].rearrange(
                            "p t j -> p (t j)"))
                for u in range(NT // 2):
                    nc.sync.dma_start(
                        out=xt[0:64, bass.ts(2 * u, 128)], in_=tmp[0:64, u, :])
                    nc.sync.dma_start(
                        out=xt[0:64, bass.ts(2 * u + 1, 128)],
                        in_=tmp[64:128, u, :])
                nc.sync.dma_start(out=xt[64:128, :], in_=xt[0:64, :])

                if stage < 2:
                    nc.sync.dma_start(
                        out=y_d[b].rearrange("(t p) j -> p t j", p=128),
                        in_=x_sb)
                    continue
                # ---- QKV projections (row-packed pairs) ----
                qk_sb = []
                for w_sb, nm in ((wq_sb, "q"), (wk_sb, "k")):
                    sb_a = qkp.tile([128, F], BF16, tag=nm + "a")
                    sb_b = qkp.tile([128, F], BF16, tag=nm + "b")
                    for fc in range(2):
                        fsl = bass.ts(fc, 512)
                        ps_a = psmm.tile([128, 512], FP32, tag="mm",
                                         name=f"qk_a_{nm}{fc}_{b}")
                        ps_b = psmm.tile([128, 512], FP32, tag="mm",
                                         name=f"qk_b_{nm}{fc}_{b}")
                        nc.tensor.matmul(
                            ps_a, w_sb[0:64, :],
                            xt[0:64, fsl], start=True, stop=True)
                        nc.tensor.matmul(
                            ps_b, w_sb[64:128, :],
                            xt[64:128, fsl], start=True, stop=True)
                        drain_copy(sb_a[:, fsl], ps_a)
                        drain_copy(sb_b[:, fsl], ps_b)
                    qk_sb.append((sb_a, sb_b))
                (qt_a, qt_b), (kt_a, kt_b) = qk_sb

                if stage < 3:
                    nc.sync.dma_start(
                        out=y_d[b].rearrange("(t p) j -> p t j", p=128),
                        in_=x_sb)
                    continue
                # v' = x @ (Wv@Wo): natural [g, (h o)=256], g-tile pairs
                # packed via row groups; one MM per PSUM bank (bank-aligned)
                vt = vp.tile([128, NT, 320], BF16, tag="v")
                nc.gpsimd.memset(vt[:, :, 256:320], 0.0)
                for gt in range(NT):
                    v_ps = psmm.tile([128, 512], FP32, tag="mm",
                                     name=f"v_ps{gt}_{b}")
                    half = gt % 2
                    nc.tensor.matmul(
                        v_ps[:, 0:256],
                        xt[bass.ds(64 * half, 64), bass.ts(gt, 128)],
                        wv_sb[bass.ds(64 * half, 64), :],
                        start=True, stop=True)
                    drain_copy(vt[:, gt, 0:256], v_ps[:, 0:256])

                if stage < 4:
                    nc.sync.dma_start(
                        out=y_d[b].rearrange("(t p) j -> p t j", p=128),
                        in_=x_sb)
                    continue
                # ---- attention: scoresT then projT accumulation ----
                # projT f-chunk accumulators [128, 512]: rows 0-63 hold the
                # real sum_h V'_h^T @ scT_h; rows 64-127 accumulate a
                # harmless byproduct of the M=128 head-pack (a matmul costs
                # N cycles regardless of M, so packing [V'_h|V'_h+1] into the
                # stationary operand halves the MM count vs M=64).
                out_f = [psacc.tile([128, 512], FP32, tag="acc",
                                    name=f"out_f{fc}_{b}")
                         for fc in range(2)]

                def emit_out_mms(hp, gt, sc0, sc1, first, last):
                    for fc in range(2):
                        # rows 0-63 += V'_{2hp}^T @ scT_{2hp}
                        nc.tensor.matmul(
                            out_f[fc][:, :],
                            vt[:, gt, bass.ds(128 * hp, 128)],
                            sc0[:, fc, :],
                            start=first, stop=False,
                            skip_group_check=True)
                        # rows 0-63 += V'_{2hp+1}^T @ scT_{2hp+1}
                        # (shifted slice: [V'_h1 | V'_h2] or [V'_h3 | 0])
                        nc.tensor.matmul(
                            out_f[fc][:, :],
                            vt[:, gt, bass.ds(128 * hp + 64, 128)],
                            sc1[:, fc, :],
                            start=False, stop=last,
                            skip_group_check=True)

                # software pipeline: defer each gt's out-MMs one iteration so
                # the in-order PE never head-of-line blocks on a score drain
                pending = None
                for hp in range(2):
                    qt = qt_a if hp == 0 else qt_b
                    kt = kt_a if hp == 0 else kt_b
                    for gt in range(NT):
                        gsl = bass.ts(gt, 128)
                        sc0 = scp.tile([128, 2, 512], BF16, tag="sc")
                        sc1 = scp.tile([128, 2, 512], BF16, tag="sc")
                        for fc in range(2):
                            fsl = bass.ts(fc, 512)
                            p0 = psmm.tile([128, 512], FP32, tag="mm",
                                           name=f"s0_{b}_{hp}_{gt}_{fc}")
                            p1 = psmm.tile([128, 512], FP32, tag="mm",
                                           name=f"s1_{b}_{hp}_{gt}_{fc}")
                            nc.tensor.matmul(
                                p0, kt[0:64, gsl], qt[0:64, fsl],
                                start=True, stop=True)
                            nc.tensor.matmul(
                                p1, kt[64:128, gsl], qt[64:128, fsl],
                                start=True, stop=True)
                            drain_relu(sc0[:, fc, :], p0)
                            drain_relu(sc1[:, fc, :], p1)
                        if pending is not None:
                            emit_out_mms(*pending)
                        pending = (hp, gt, sc0, sc1,
                                   hp == 0 and gt == 0,
                                   hp == 1 and gt == NT - 1)
                emit_out_mms(*pending)

                if stage < 5:
                    nc.sync.dma_start(
                        out=y_d[b].rearrange("(t p) j -> p t j", p=128),
                        in_=x_sb)
                    continue
                # ---- projT -> natural + residual + LayerNorm ----
                pj = pjp.tile([64, 2, 512], BF16, tag="pj")
                drain_copy(pj[:, 0, :], out_f[0][0:64, :])
                drain_copy(pj[:, 1, :], out_f[1][0:64, :])
                nat_sb = resp.tile([128, NT, DIN], BF16, tag="natsb")
                for t in range(NT):
                    fc, tw = divmod(t, 4)
                    nc.sync.dma_start_transpose(
                        out=nat_sb[:, t, :], in_=pj[:, fc, bass.ts(tw, 128)])
                res = resp.tile([128, NT, DIN], FP32, tag="res")
                nc.vector.tensor_add(out=res, in0=nat_sb, in1=x_res)

                sq = resp.tile([128, NT, DIN], FP32, tag="sq")
                nc.gpsimd.tensor_mul(out=sq, in0=res, in1=res)
                stat = statp.tile([128, NT, 2], FP32, tag="stat")
                nc.vector.tensor_reduce(
                    out=stat[:, :, 0], in_=res,
                    axis=mybir.AxisListType.X, op=mybir.AluOpType.add)
                nc.vector.tensor_reduce(
                    out=stat[:, :, 1], in_=sq,
                    axis=mybir.AxisListType.X, op=mybir.AluOpType.add)
                mv = statp.tile([128, NT, 4], FP32, tag="mv")
                # mean, E[x^2]
                nc.vector.tensor_scalar_mul(
                    out=mv[:, :, 0], in0=stat[:, :, 0], scalar1=1.0 / DIN)
                nc.vector.tensor_scalar_mul(
                    out=mv[:, :, 1], in0=stat[:, :, 1], scalar1=1.0 / DIN)
                # var = E[x^2] - mean^2
                nc.vector.tensor_mul(
                    out=mv[:, :, 2], in0=mv[:, :, 0], in1=mv[:, :, 0])
                nc.vector.tensor_sub(
                    out=mv[:, :, 2], in0=mv[:, :, 1], in1=mv[:, :, 2])
                # rstd = 1/sqrt(var + eps)
                nc.scalar.activation(
                    out=mv[:, :, 3], in_=mv[:, :, 2],
                    func=mybir.ActivationFunctionType.Sqrt, bias=eps_sb)
                nc.vector.reciprocal(out=mv[:, :, 3], in_=mv[:, :, 3])

                o_sb = resp.tile([128, NT, DIN], FP32, tag="o")
                for t in range(NT):
                    nc.vector.tensor_scalar(
                        out=o_sb[:, t, :], in0=res[:, t, :],
                        scalar1=mv[:, t, 0:1], scalar2=mv[:, t, 3:4],
                        op0=mybir.AluOpType.subtract,
                        op1=mybir.AluOpType.mult)
                if use_gb:
                    nc.gpsimd.tensor_mul(out=o_sb, in0=o_sb, in1=g_rep)
                    nc.gpsimd.tensor_add(out=o_sb, in0=o_sb, in1=b_rep)
                nc.sync.dma_start(
                    out=y_d[b].rearrange("(t p) j -> p t j", p=128), in_=o_sb)

    split_multiwaits(nc)
    return nc


def kernel(featureVec, Wqkv, Wo, bo, ln_gamma, ln_beta):
    x = np.ascontiguousarray(np.asarray(featureVec, dtype=np.float32))
    Wqkv = np.asarray(Wqkv, dtype=np.float32)
    Wo = np.asarray(Wo, dtype=np.float32)
    bo = np.asarray(bo, dtype=np.float32)
    g = np.asarray(ln_gamma, dtype=np.float32)
    be = np.asarray(ln_beta, dtype=np.float32)

    # host-side weight packing / folding
    wq_pack = np.concatenate([Wqkv[h, 0] * 0.125 for h in range(H)], axis=1)
    wk_pack = np.concatenate([Wqkv[h, 1] for h in range(H)], axis=1)
    wv_pack = np.concatenate(
        [(Wqkv[h, 2].astype(np.float64)
          @ Wo[h * DOUT:(h + 1) * DOUT].astype(np.float64)).astype(np.float32)
         for h in range(H)], axis=1)
    import ml_dtypes
    bf = ml_dtypes.bfloat16
    wq_host = np.ascontiguousarray(
        np.concatenate([wq_pack[:, 0:128], wq_pack[:, 128:256]],
                       axis=0).astype(bf))
    wk_host = np.ascontiguousarray(
        np.concatenate([wk_pack[:, 0:128], wk_pack[:, 128:256]],
                       axis=0).astype(bf))
    wv_host = np.ascontiguousarray(
        np.concatenate([wv_pack, wv_pack], axis=0).astype(bf))

    use_gb = not (np.all(g == 1.0) and np.all(be == 0.0))
    use_bo = not np.all(bo == 0.0)

    key = (use_gb, use_bo)
    if key not in _cache:
        _cache[key] = _build(use_gb, use_bo)
    nc = _cache[key]

    in_maps = []
    for c in range(NCORES):
        m = {
            "x": np.ascontiguousarray(x[c * BPC:(c + 1) * BPC]),
            "wq": wq_host, "wk": wk_host, "wv": wv_host,
        }
        if use_gb:
            m["gb"] = np.ascontiguousarray(np.stack([g, be]))
        if use_bo:
            m["bo"] = bo
        in_maps.append(m)

    res = run_bass_kernel_spmd(nc, in_maps, core_ids=list(range(NCORES)))
    return np.concatenate([r["y"] for r in res.results], axis=0)


if __name__ == "__main__":
    rng = np.random.default_rng(0)
    inputs = {
        "featureVec": rng.standard_normal((B, F, DIN), dtype=np.float32),
        "Wqkv": (rng.standard_normal((H, 3, DIN, DOUT), dtype=np.float32)
                 / np.sqrt(DIN).astype(np.float32)),
        "Wo": (rng.standard_normal((H * DOUT, DIN), dtype=np.float32)
               / np.sqrt(H * DOUT).astype(np.float32)),
        "bo": np.zeros(DIN, np.float32),
        "ln_gamma": np.ones(DIN, np.float32),
        "ln_beta": np.zeros(DIN, np.float32),
    }
    out = kernel(**inputs)
    print(out.shape, out.dtype, float(np.abs(out).max()))


# revision 34
# speedup vs baseline: 1.1090x; 1.0033x over previous
"""Trainium2 Bass kernel for a multi-head ReLU-attention transformer layer.

Shapes (hardcoded): B=32, F=1024, DIN=64, DOUT=64, H=4.
  qkv   = einsum("bfi,hkio->bhkfo", x, Wqkv)
  scores= relu(q @ k^T / sqrt(DOUT))
  head  = scores @ v
  out   = LN(concat(head) @ Wo + bo + x) * gamma + beta

Sharding: pure data-parallel over batch B across 8 NeuronCores (4 b/core).

Host-side algebraic folds (exact or fp32-precise):
  - 1/sqrt(DOUT)=0.125 folded into Wq (exact, power of two).
  - Wo folded into Wv:  proj = sum_h scores_h @ (Wv_h @ Wo_h).

Per-batch device pipeline (all matmuls bf16 with fp32 PSUM accumulation —
fp32/fp32r matmuls silently return zeros on this toolchain):
  x -> (bf16 cast, DMA-xbar transpose) xT, duplicated onto both partition
  halves so 64-deep contractions pack two-per-MM via PE row groups.
  Q^T/K^T per head-pair land stacked on partition halves; scoresT =
  relu(K^T_tile^T @ Q^T) drains PSUM->SBUF via ScalarE/VectorE (the
  bandwidth-critical path: PSUM fp32 reads are capped at 1 elem/lane/cycle);
  projT accumulates over heads and g-tiles into two [64,512] PSUM banks
  (matmul PSUM outputs must be bank-aligned on this hardware); DMA-xbar
  transposes back to natural layout; residual + LayerNorm in fp32; DMA out.

This walrus build accepts only ONE sync wait per instruction; Tile emits
multi-waits, so split_multiwaits() hoists extras onto NoOps post-schedule.
"""

import numpy as np

import concourse.bass as bass
import concourse.mybir as mybir
import concourse.tile as tile
from concourse.bass_utils import run_bass_kernel_spmd


def split_multiwaits(nc):
    """Hoist all but the last sync wait of any instruction onto standalone
    NoOps inserted just before it on the same engine — semantically identical
    (same-engine program order runs the waits first), but keeps every
    instruction within this walrus build's one-wait limit."""
    n_split = 0
    max_upd = 0

    def fix_block(bl):
        nonlocal n_split, max_upd
        insts = list(bl.instructions)
        out = []
        changed = False
        for inst in insts:
            si = inst.sync_info
            if si is not None:
                max_upd = max(max_upd, len(si.on_update))
                waits = list(si.on_wait)
                if len(waits) > 1:
                    for k, w in enumerate(waits[:-1]):
                        nop = mybir.InstNoOp(
                            name=f"{inst.name}-wsplit{k}", ins=[], outs=[])
                        nop.engine = inst.engine
                        nop.sync_info = mybir.SyncInfo(
                            on_wait=[w], on_update=[])
                        out.append(nop)
                    inst.sync_info = mybir.SyncInfo(
                        on_wait=[waits[-1]], on_update=list(si.on_update))
                    n_split += 1
                    changed = True
            out.append(inst)
        if changed:
            bl.instructions = out
        for sub in getattr(bl, "blocks", None) or []:
            fix_block(sub)

    for f in nc.m.functions:
        for bl in f.blocks:
            fix_block(bl)
    assert max_upd <= 1, f"need update-splitting too: {max_upd}"
    return n_split


B, F, DIN, DOUT, H = 32, 1024, 64, 64, 4
NCORES = 8
BPC = B // NCORES  # batches per core
NT = F // 128  # 8 f-tiles per batch
FP32 = mybir.dt.float32
BF16 = mybir.dt.bfloat16
EPS = 1e-5

_cache = {}


def _build(use_gb: bool, use_bo: bool, stage: int = 99):
    nc = bass.Bass("TRN2", target_bir_lowering=False, debug=False,
                   num_devices=NCORES)
    x_d = nc.dram_tensor("x", [BPC, F, DIN], FP32, kind="ExternalInput").ap()
    wq_d = nc.dram_tensor("wq", [128, 128], BF16, kind="ExternalInput").ap()
    wk_d = nc.dram_tensor("wk", [128, 128], BF16, kind="ExternalInput").ap()
    wv_d = nc.dram_tensor("wv", [128, 256], BF16, kind="ExternalInput").ap()
    if use_gb:
        gb_d = nc.dram_tensor("gb", [2, DIN], FP32, kind="ExternalInput").ap()
    if use_bo:
        bo_d = nc.dram_tensor("bo", [DIN], FP32, kind="ExternalInput").ap()
    y_d = nc.dram_tensor("y", [BPC, F, DIN], FP32, kind="ExternalOutput").ap()

    # alternate score drains between ScalarE and VectorE, weighted toward
    # ScalarE (1.2 GHz vs 0.96 GHz): ACT gets 8 of every 15 (equal-time
    # balance point measured via the cost model)
    drain_pat = ([True, False] * 7 + [True])
    drain_i = [0]

    def drain_relu(out_ap, in_ap):
        use_act = drain_pat[drain_i[0] % len(drain_pat)]
        drain_i[0] += 1
        if use_act:
            nc.scalar.activation(out=out_ap, in_=in_ap,
                                 func=mybir.ActivationFunctionType.Relu)
        else:
            nc.vector.tensor_scalar_max(out=out_ap, in0=in_ap, scalar1=0.0)

    def drain_copy(out_ap, in_ap, act=None):
        if act is None:
            act = drain_pat[drain_i[0] % len(drain_pat)]
            drain_i[0] += 1
        if act:
            nc.scalar.activation(out=out_ap, in_=in_ap,
                                 func=mybir.ActivationFunctionType.Copy)
        else:
            nc.vector.tensor_copy(out=out_ap, in_=in_ap)

    with tile.TileContext(nc) as tc:
        with (
            tc.tile_pool(name="const", bufs=1) as constp,
            tc.tile_pool(name="xp", bufs=3) as xp,
            tc.tile_pool(name="xtp", bufs=3) as xtp,
            tc.tile_pool(name="qkp", bufs=3) as qkp,
            tc.tile_pool(name="vp", bufs=3) as vp,
            tc.tile_pool(name="scp", bufs=24) as scp,
            tc.tile_pool(name="pjp", bufs=3) as pjp,
            tc.tile_pool(name="resp", bufs=3) as resp,
            tc.tile_pool(name="statp", bufs=4) as statp,
            tc.tile_pool(name="mm", bufs=6, space="PSUM") as psmm,
            tc.tile_pool(name="acc", bufs=2, space="PSUM") as psacc,
        ):
            # ---- constants ----
            eps_sb = constp.tile([128, 1], FP32)
            nc.vector.memset(eps_sb, EPS)
            wq_sb = constp.tile([128, 128], BF16)
            nc.sync.dma_start(out=wq_sb, in_=wq_d)
            wk_sb = constp.tile([128, 128], BF16)
            nc.sync.dma_start(out=wk_sb, in_=wk_d)
            wv_sb = constp.tile([128, 256], BF16)
            nc.sync.dma_start(out=wv_sb, in_=wv_d)
            if use_gb:
                g_rep = constp.tile([128, NT, DIN], FP32)
                b_rep = constp.tile([128, NT, DIN], FP32)
                for t in range(NT):
                    nc.gpsimd.dma_start(
                        out=g_rep[:, t, :],
                        in_=bass.AP(gb_d.tensor, 0, [[0, 128], [1, DIN]]))
                    nc.gpsimd.dma_start(
                        out=b_rep[:, t, :],
                        in_=bass.AP(gb_d.tensor, DIN, [[0, 128], [1, DIN]]))
            if use_bo:
                bo_rep = constp.tile([128, DIN], FP32)
                nc.gpsimd.dma_start(
                    out=bo_rep,
                    in_=bass.AP(bo_d.tensor, 0, [[0, 128], [1, DIN]]))

            for b in range(BPC):
                # ---- load x (natural: partition = f within tile) ----
                x_sb = xp.tile([128, NT, DIN], FP32, tag="x")
                nc.sync.dma_start(
                    out=x_sb, in_=x_d[b].rearrange("(t p) j -> p t j", p=128))
                if use_bo:
                    x_res = xp.tile([128, NT, DIN], FP32, tag="xres")
                    for t in range(NT):
                        nc.vector.tensor_add(
                            out=x_res[:, t, :], in0=x_sb[:, t, :], in1=bo_rep)
                else:
                    x_res = x_sb
                x_bf = xp.tile([128, NT, DIN], BF16, tag="xbf")
                nc.vector.tensor_copy(out=x_bf, in_=x_sb)

                # ---- transpose x -> xT [64, 1024] via DMA xbar, dup ----
                # xbar tiles are 16x128, so transpose f-tile PAIRS as
                # [128,128] blocks: top half = xT of even tile, bottom = odd.
                # All transposes issue before all copies: every
                # DMATranspose<->DMACopy xbar-mode transition serializes the
                # DMA path on this hardware, so batch the modes.
                xt = xtp.tile([128, F], BF16, tag="xt")
                tmp = xtp.tile([128, NT // 2, 128], BF16, tag="tmpt")
                for u in range(NT // 2):
                    nc.sync.dma_start_transpose(
                        out=tmp[:, u, :],
                        in_=x_bf[:, 2 * u:2 * u + 2, :].rearrange(
                            "p t j -> p (t j)"))
                for u in range(NT // 2):
                    nc.sync.dma_start(
                        out=xt[0:64, bass.ts(2 * u, 128)], in_=tmp[0:64, u, :])
                    nc.sync.dma_start(
                        out=xt[0:64, bass.ts(2 * u + 1, 128)],
                        in_=tmp[64:128, u, :])
                nc.sync.dma_start(out=xt[64:128, :], in_=xt[0:64, :])

                if stage < 2:
                    nc.sync.dma_start(
                        out=y_d[b].rearrange("(t p) j -> p t j", p=128),
                        in_=x_sb)
                    continue
                # ---- QKV projections (row-packed pairs) ----
                qk_sb = []
                for w_sb, nm in ((wq_sb, "q"), (wk_sb, "k")):
                    sb_a = qkp.tile([128, F], BF16, tag=nm + "a")
                    sb_b = qkp.tile([128, F], BF16, tag=nm + "b")
                    for fc in range(2):
                        fsl = bass.ts(fc, 512)
                        ps_a = psmm.tile([128, 512], FP32, tag="mm",
                                         name=f"qk_a_{nm}{fc}_{b}")
                        ps_b = psmm.tile([128, 512], FP32, tag="mm",
                                         name=f"qk_b_{nm}{fc}_{b}")
                        nc.tensor.matmul(
                            ps_a, w_sb[0:64, :],
                            xt[0:64, fsl], start=True, stop=True)
                        nc.tensor.matmul(
                            ps_b, w_sb[64:128, :],
                            xt[64:128, fsl], start=True, stop=True)
                        drain_copy(sb_a[:, fsl], ps_a)
                        drain_copy(sb_b[:, fsl], ps_b)
                    qk_sb.append((sb_a, sb_b))
                (qt_a, qt_b), (kt_a, kt_b) = qk_sb

                if stage < 3:
                    nc.sync.dma_start(
                        out=y_d[b].rearrange("(t p) j -> p t j", p=128),
                        in_=x_sb)
                    continue
                # v' = x @ (Wv@Wo): natural [g, (h o)=256], g-tile pairs
                # packed via row groups; one MM per PSUM bank (bank-aligned)
                vt = vp.tile([128, NT, 320], BF16, tag="v")
                nc.gpsimd.memset(vt[:, :, 256:320], 0.0)
                for gt in range(NT):
                    v_ps = psmm.tile([128, 512], FP32, tag="mm",
                                     name=f"v_ps{gt}_{b}")
                    half = gt % 2
                    nc.tensor.matmul(
                        v_ps[:, 0:256],
                        xt[bass.ds(64 * half, 64), bass.ts(gt, 128)],
                        wv_sb[bass.ds(64 * half, 64), :],
                        start=True, stop=True)
                    drain_copy(vt[:, gt, 0:256], v_ps[:, 0:256])

                if stage < 4:
                    nc.sync.dma_start(
                        out=y_d[b].rearrange("(t p) j -> p t j", p=128),
                        in_=x_sb)
                    continue
                # ---- attention: scoresT then projT accumulation ----
                # projT f-chunk accumulators [128, 512]: rows 0-63 hold the
                # real sum_h V'_h^T @ scT_h; rows 64-127 accumulate a
                # harmless byproduct of the M=128 head-pack (a matmul costs
                # N cycles regardless of M, so packing [V'_h|V'_h+1] into the
                # stationary operand halves the MM count vs M=64).
                out_f = [psacc.tile([128, 512], FP32, tag="acc",
                                    name=f"out_f{fc}_{b}")
                         for fc in range(2)]

                def emit_out_mms(hp, gt, sc0, sc1, first, last):
                    for fc in range(2):
                        # rows 0-63 += V'_{2hp}^T @ scT_{2hp}
                        nc.tensor.matmul(
                            out_f[fc][:, :],
                            vt[:, gt, bass.ds(128 * hp, 128)],
                            sc0[fc],
                            start=first, stop=False,
                            skip_group_check=True)
                        # rows 0-63 += V'_{2hp+1}^T @ scT_{2hp+1}
                        # (shifted slice: [V'_h1 | V'_h2] or [V'_h3 | 0])
                        nc.tensor.matmul(
                            out_f[fc][:, :],
                            vt[:, gt, bass.ds(128 * hp + 64, 128)],
                            sc1[fc],
                            start=False, stop=last,
                            skip_group_check=True)

                # software pipeline: defer each gt's out-MMs one iteration so
                # the in-order PE never head-of-line blocks on a score drain
                pending = None
                for hp in range(2):
                    qt = qt_a if hp == 0 else qt_b
                    kt = kt_a if hp == 0 else kt_b
                    for gt in range(NT):
                        gsl = bass.ts(gt, 128)
                        sc0 = [scp.tile([128, 512], BF16, tag="sc",
                                        name=f"sc0_{b}_{hp}_{gt}_{f}")
                               for f in range(2)]
                        sc1 = [scp.tile([128, 512], BF16, tag="sc",
                                        name=f"sc1_{b}_{hp}_{gt}_{f}")
                               for f in range(2)]
                        for fc in range(2):
                            fsl = bass.ts(fc, 512)
                            p0 = psmm.tile([128, 512], FP32, tag="mm",
                                           name=f"s0_{b}_{hp}_{gt}_{fc}")
                            p1 = psmm.tile([128, 512], FP32, tag="mm",
                                           name=f"s1_{b}_{hp}_{gt}_{fc}")
                            nc.tensor.matmul(
                                p0, kt[0:64, gsl], qt[0:64, fsl],
                                start=True, stop=True)
                            nc.tensor.matmul(
                                p1, kt[64:128, gsl], qt[64:128, fsl],
                                start=True, stop=True)
                            drain_relu(sc0[fc], p0)
                            drain_relu(sc1[fc], p1)
                        if pending is not None:
                            emit_out_mms(*pending)
                        pending = (hp, gt, sc0, sc1,
                                   hp == 0 and gt == 0,
                                   hp == 1 and gt == NT - 1)
                emit_out_mms(*pending)

                if stage < 5:
                    nc.sync.dma_start(
                        out=y_d[b].rearrange("(t p) j -> p t j", p=128),
                        in_=x_sb)
                    continue
                # ---- projT -> natural + residual + LayerNorm ----
                pj = pjp.tile([64, 2, 512], BF16, tag="pj")
                drain_copy(pj[:, 0, :], out_f[0][0:64, :])
                drain_copy(pj[:, 1, :], out_f[1][0:64, :])
                nat_sb = resp.tile([128, NT, DIN], BF16, tag="natsb")
                for t in range(NT):
                    fc, tw = divmod(t, 4)
                    nc.sync.dma_start_transpose(
                        out=nat_sb[:, t, :], in_=pj[:, fc, bass.ts(tw, 128)])
                res = resp.tile([128, NT, DIN], FP32, tag="res")
                nc.vector.tensor_add(out=res, in0=nat_sb, in1=x_res)

                sq = resp.tile([128, NT, DIN], FP32, tag="sq")
                nc.gpsimd.tensor_mul(out=sq, in0=res, in1=res)
                stat = statp.tile([128, NT, 2], FP32, tag="stat")
                nc.vector.tensor_reduce(
                    out=stat[:, :, 0], in_=res,
                    axis=mybir.AxisListType.X, op=mybir.AluOpType.add)
                nc.vector.tensor_reduce(
                    out=stat[:, :, 1], in_=sq,
                    axis=mybir.AxisListType.X, op=mybir.AluOpType.add)
                mv = statp.tile([128, NT, 4], FP32, tag="mv")
                # mean, E[x^2]
                nc.vector.tensor_scalar_mul(
                    out=mv[:, :, 0], in0=stat[:, :, 0], scalar1=1.0 / DIN)
                nc.vector.tensor_scalar_mul(
                    out=mv[:, :, 1], in0=stat[:, :, 1], scalar1=1.0 / DIN)
                # var = E[x^2] - mean^2
                nc.vector.tensor_mul(
                    out=mv[:, :, 2], in0=mv[:, :, 0], in1=mv[:, :, 0])
                nc.vector.tensor_sub(
                    out=mv[:, :, 2], in0=mv[:, :, 1], in1=mv[:, :, 2])
                # rstd = 1/sqrt(var + eps)
                nc.scalar.activation(
                    out=mv[:, :, 3], in_=mv[:, :, 2],
                    func=mybir.ActivationFunctionType.Sqrt, bias=eps_sb)
                nc.vector.reciprocal(out=mv[:, :, 3], in_=mv[:, :, 3])

                o_sb = resp.tile([128, NT, DIN], FP32, tag="o")
                for t in range(NT):
                    nc.vector.tensor_scalar(
                        out=o_sb[:, t, :], in0=res[:, t, :],
                        scalar1=mv[:, t, 0:1], scalar2=mv[:, t, 3:4],
                        op0=mybir.AluOpType.subtract,
                        op1=mybir.AluOpType.mult)
                if use_gb:
                    nc.gpsimd.tensor_mul(out=o_sb, in0=o_sb, in1=g_rep)
                    nc.gpsimd.tensor_add(out=o_sb, in0=o_sb, in1=b_rep)
                nc.sync.dma_start(
                    out=y_d[b].rearrange("(t p) j -> p t j", p=128), in_=o_sb)

    split_multiwaits(nc)
    return nc


def kernel(featureVec, Wqkv, Wo, bo, ln_gamma, ln_beta):
    x = np.ascontiguousarray(np.asarray(featureVec, dtype=np.float32))
    Wqkv = np.asarray(Wqkv, dtype=np.float32)
    Wo = np.asarray(Wo, dtype=np.float32)
    bo = np.asarray(bo, dtype=np.float32)
    g = np.asarray(ln_gamma, dtype=np.float32)
    be = np.asarray(ln_beta, dtype=np.float32)

    # host-side weight packing / folding
    wq_pack = np.concatenate([Wqkv[h, 0] * 0.125 for h in range(H)], axis=1)
    wk_pack = np.concatenate([Wqkv[h, 1] for h in range(H)], axis=1)
    wv_pack = np.concatenate(
        [(Wqkv[h, 2].astype(np.float64)
          @ Wo[h * DOUT:(h + 1) * DOUT].astype(np.float64)).astype(np.float32)
         for h in range(H)], axis=1)
    import ml_dtypes
    bf = ml_dtypes.bfloat16
    wq_host = np.ascontiguousarray(
        np.concatenate([wq_pack[:, 0:128], wq_pack[:, 128:256]],
                       axis=0).astype(bf))
    wk_host = np.ascontiguousarray(
        np.concatenate([wk_pack[:, 0:128], wk_pack[:, 128:256]],
                       axis=0).astype(bf))
    wv_host = np.ascontiguousarray(
        np.concatenate([wv_pack, wv_pack], axis=0).astype(bf))

    use_gb = not (np.all(g == 1.0) and np.all(be == 0.0))
    use_bo = not np.all(bo == 0.0)

    key = (use_gb, use_bo)
    if key not in _cache:
        _cache[key] = _build(use_gb, use_bo)
    nc = _cache[key]

    in_maps = []
    for c in range(NCORES):
        m = {
            "x": np.ascontiguousarray(x[c * BPC:(c + 1) * BPC]),
            "wq": wq_host, "wk": wk_host, "wv": wv_host,
        }
        if use_gb:
            m["gb"] = np.ascontiguousarray(np.stack([g, be]))
        if use_bo:
            m["bo"] = bo
        in_maps.append(m)

    res = run_bass_kernel_spmd(nc, in_maps, core_ids=list(range(NCORES)))
    return np.concatenate([r["y"] for r in res.results], axis=0)


if __name__ == "__main__":
    rng = np.random.default_rng(0)
    inputs = {
        "featureVec": rng.standard_normal((B, F, DIN), dtype=np.float32),
        "Wqkv": (rng.standard_normal((H, 3, DIN, DOUT), dtype=np.float32)
                 / np.sqrt(DIN).astype(np.float32)),
        "Wo": (rng.standard_normal((H * DOUT, DIN), dtype=np.float32)
               / np.sqrt(H * DOUT).astype(np.float32)),
        "bo": np.zeros(DIN, np.float32),
        "ln_gamma": np.ones(DIN, np.float32),
        "ln_beta": np.zeros(DIN, np.float32),
    }
    out = kernel(**inputs)
    print(out.shape, out.dtype, float(np.abs(out).max()))
